# revision 1
# baseline (speedup 1.0000x reference)
"""Trainium2 Bass kernel for DFine multi-head attention.

Problem: B=2, S=2048, D=1024, H=16 heads, HD=64.
Sharding over 8 cores: core c handles batch b=c//4 and head-group g=c%4
(4 heads). Each core computes its heads' attention and a partial
out-projection [2048, 1024]; the host sums the 4 partials per batch and
adds the output bias.

All matmuls run in float32r (TF32-like, full PE rate for moving dim
>= 256, ~1.5e-4 relative error).
"""

import sys
import numpy as np

if "/opt/trn_rl_repo" not in sys.path:
    sys.path.insert(0, "/opt/trn_rl_repo")

B, S, D, H, HD = 2, 2048, 1024, 16, 64
G = 4          # heads per core
E = G * HD     # 256 per-core head width
T = S          # tokens
KC = 8         # contraction chunks of 128 over D
TB = 512       # t-block (moving free dim)
NT = T // TB   # 4
NS = T // 128  # 16 s-chunks
SCALE = HD ** -0.5

_PROGRAM = None


def _build_program(reps=1):
    import concourse.bacc as bacc
    import concourse.tile as tile
    from concourse import mybir

    f32 = mybir.dt.float32

    nc = bacc.Bacc("TRN2", target_bir_lowering=False, debug=False)

    xT_d = nc.declare_dram_parameter("xT", [D, T], f32, isOutput=False)
    pT_d = nc.declare_dram_parameter("pT", [D, T], f32, isOutput=False)
    wq_d = nc.declare_dram_parameter("wq", [D, E], f32, isOutput=False)
    wk_d = nc.declare_dram_parameter("wk", [D, E], f32, isOutput=False)
    wv_d = nc.declare_dram_parameter("wv", [D, E], f32, isOutput=False)
    wo_d = nc.declare_dram_parameter("wo", [E, D], f32, isOutput=False)
    bq_d = nc.declare_dram_parameter("bq", [2, 128, 1], f32, isOutput=False)
    bk_d = nc.declare_dram_parameter("bk", [2, 128, 1], f32, isOutput=False)
    bv_d = nc.declare_dram_parameter("bvr", [128, E], f32, isOutput=False)
    out_d = nc.declare_dram_parameter("out", [T, D], f32, isOutput=True)

    with tile.TileContext(nc) as tc:
        for rep in range(reps):
            _build_body(nc, tc, mybir, rep,
                        (xT_d, pT_d, wq_d, wk_d, wv_d, wo_d, bq_d, bk_d,
                         bv_d, out_d))

    nc.compile()
    return nc


def _build_body(nc, tc, mybir, rep, drams):
    from contextlib import ExitStack

    fr = mybir.dt.float32r
    f32 = mybir.dt.float32
    Exp = mybir.ActivationFunctionType.Exp
    (xT_d, pT_d, wq_d, wk_d, wv_d, wo_d, bq_d, bk_d, bv_d, out_d) = drams
    R = f"r{rep}_"

    octx = ExitStack()
    wpool = octx.enter_context(tc.tile_pool(name=f"{R}wpool", bufs=1))
    qkpool = octx.enter_context(tc.tile_pool(name=f"{R}qkpool", bufs=1))
    vpool = octx.enter_context(tc.tile_pool(name=f"{R}vpool", bufs=1))

    # ---- persistent tiles ----
    wq_t = wpool.tile([128, KC, E], fr, name=f"{R}wq_t")
    wk_t = wpool.tile([128, KC, E], fr, name=f"{R}wk_t")
    wv_t = wpool.tile([128, KC, E], fr, name=f"{R}wv_t")
    bq_t = wpool.tile([128, 2, 1], f32, name=f"{R}bq_t")
    bk_t = wpool.tile([128, 2, 1], f32, name=f"{R}bk_t")
    bv_t = wpool.tile([128, E], f32, name=f"{R}bv_t")
    ones_f = wpool.tile([1, 64], f32, name=f"{R}ones_f")
    ones_r = wpool.tile([1, 64], fr, name=f"{R}ones_r")
    oneblk = wpool.tile([128, NS, G, 1], f32, name=f"{R}oneblk")

    qT = [qkpool.tile([128, T], fr, name=f"{R}qT{p}") for p in range(2)]
    kT = [qkpool.tile([128, T], fr, name=f"{R}kT{p}") for p in range(2)]
    v_aug = vpool.tile([128, NS, G, HD + 1], fr, name=f"{R}v_aug")

    # ---- weight / bias DMAs (first: v-proj needs wv immediately) ----
    nc.gpsimd.dma_start(
        wv_t[:], wv_d[:].bitcast(fr).rearrange("(c p) e -> p c e", p=128))
    nc.gpsimd.dma_start(
        wq_t[:], wq_d[:].bitcast(fr).rearrange("(c p) e -> p c e", p=128))
    nc.gpsimd.dma_start(
        wk_t[:], wk_d[:].bitcast(fr).rearrange("(c p) e -> p c e", p=128))
    nc.gpsimd.dma_start(bq_t[:], bq_d[:].rearrange("c p o -> p c o"))
    nc.gpsimd.dma_start(bk_t[:], bk_d[:].rearrange("c p o -> p c o"))
    nc.gpsimd.dma_start(bv_t[:], bv_d[:])
    nc.vector.memset(ones_f[:], 1.0)
    nc.vector.tensor_copy(ones_r[:], ones_f[:])
    nc.vector.memset(oneblk[:], 1.0)
    nc.vector.tensor_copy(v_aug[:, :, :, HD:HD + 1], oneblk[:])

    # ---- phase A/B: projections (DMA-overlapped, k-outer) ----
    ictx = ExitStack()
    ppool = ictx.enter_context(tc.tile_pool(name=f"{R}ppool", bufs=1))
    hT_t = ppool.tile([128, KC, T], fr, name=f"{R}hT_t")
    qkps = ictx.enter_context(tc.tile_pool(name=f"{R}qkps", bufs=1,
                                           space="PSUM"))

    actx = ExitStack()
    xpool = actx.enter_context(tc.tile_pool(name=f"{R}xpool", bufs=1))
    pps = actx.enter_context(tc.tile_pool(name=f"{R}pps", bufs=1,
                                          space="PSUM"))
    xT_t = xpool.tile([128, KC, T], fr, name=f"{R}xT_t")
    for k in range(KC):
        nc.sync.dma_start(
            xT_t[:, k, :], xT_d[:].bitcast(fr)[k * 128:(k + 1) * 128, :])
        nc.sync.dma_start(
            hT_t[:, k, :], pT_d[:].bitcast(fr)[k * 128:(k + 1) * 128, :])

    # hT = xT + pT in place on the pT tiles (gated only by the two DMAs);
    # two half-adds per chunk so the first q/k k-step unblocks sooner
    for k in range(KC):
        for hf in range(2):
            sl = slice(hf * (T // 2), (hf + 1) * (T // 2))
            nc.vector.tensor_tensor(
                hT_t[:, k, sl], hT_t[:, k, sl], xT_t[:, k, sl],
                op=mybir.AluOpType.add)

    # q/k projections, k-outer. Pair-0 (the phase-C critical path) gets
    # all 8 concurrent psum groups: q in pps tags 0-3, k in pps 4-5 +
    # the two long-lived qkps tags. Pair-1 is emitted mid-phase-C.
    def qk_wave(w_t, b_t, dsts, nm, p, tbs, slots=None):
        pss = {}
        for i, tb in enumerate(tbs):
            if slots is None:
                pss[tb] = qkps.tile([128, TB], f32,
                                    name=f"{R}{nm}ps{p}{tb}",
                                    tag=f"qk{tb % 2}")
            else:
                pool, tag = slots[i]
                pss[tb] = pool.tile([128, TB], f32,
                                    name=f"{R}{nm}ps{p}{tb}", tag=tag)
        for k in range(KC):
            for tb in tbs:
                nc.tensor.matmul(
                    pss[tb][:],
                    w_t[:, k, p * 128:(p + 1) * 128],
                    hT_t[:, k, tb * TB:(tb + 1) * TB],
                    start=(k == 0), stop=(k == KC - 1))
        for tb in tbs:
            nc.scalar.activation(
                dsts[p][:, tb * TB:(tb + 1) * TB], pss[tb][:],
                mybir.ActivationFunctionType.Identity, bias=b_t[:, p, :])

    # v projection emission happens in phase C (after attention_pair(0,0))
    # so its matmuls fill PE under the ACT-bound stretch; defined here for
    # access to xT/wv tiles. 4-chunk psum windows on the 2 qkps banks,
    # si-outer so slice si completes just ahead of attnV's demand.
    def v_proj():
        for si in range(NS):
            for w in range(2):
                ps = qkps.tile([128, E], f32, name=f"{R}vp{w}_{si}",
                               tag=f"qk{w}")
                for kk in range(4):
                    k = w * 4 + kk
                    nc.tensor.matmul(
                        ps[:], xT_t[:, k, si * 128:(si + 1) * 128],
                        wv_t[:, k, :], start=(kk == 0), stop=(kk == 3))
                dst = v_aug[:, si, :, 0:HD]
                psg = ps[:].rearrange("p (g e) -> p g e", g=G)
                if w == 0:
                    nc.vector.tensor_tensor(
                        dst, psg, bv_t[:].rearrange("p (g e) -> p g e", g=G),
                        op=mybir.AluOpType.add)
                else:
                    nc.vector.tensor_tensor(dst, dst, psg,
                                            op=mybir.AluOpType.add)

    v_proj()
    qk_wave(wq_t, bq_t, qT, "q", 0, (0, 1, 2, 3),
            slots=[(pps, f"t{i}") for i in range(4)])
    qk_wave(wk_t, bk_t, kT, "k", 0, (0, 1, 2, 3),
            slots=[(pps, "t4"), (pps, "t5"), (qkps, "qk0"), (qkps, "qk1")])
    actx.close()  # frees xT + the 6-bank pair-0 psum pool

    # ---- phase C/D: attention + out-projection ----
    cctx = ExitStack()
    a2pool = cctx.enter_context(tc.tile_pool(name=f"{R}a2pool", bufs=1))
    epool = cctx.enter_context(tc.tile_pool(name=f"{R}epool", bufs=7))
    npool = cctx.enter_context(tc.tile_pool(name=f"{R}npool", bufs=2))
    opool = cctx.enter_context(tc.tile_pool(name=f"{R}opool", bufs=2))
    scps = cctx.enter_context(tc.tile_pool(name=f"{R}scps", bufs=2,
                                           space="PSUM"))
    atps = cctx.enter_context(tc.tile_pool(name=f"{R}atps", bufs=1,
                                           space="PSUM"))

    at2 = [a2pool.tile([128, T], fr, name=f"{R}at2_{p}") for p in range(2)]
    wo_t = a2pool.tile([128, 2, D], fr, name=f"{R}wo_t")
    nc.gpsimd.dma_start(
        wo_t[:], wo_d[:].bitcast(fr).rearrange("(c p) d -> p c d", p=128))

    def attention_pair(tb, p):
        t0 = tb * TB
        atp = [atps.tile([HD + 1, TB], f32, name=f"{R}at_{tb}_{p}_{h}",
                         tag=f"at{h}") for h in range(2)]
        for si in range(NS):
            scp = scps.tile([128, 2, TB], f32,
                            name=f"{R}sc_{tb}_{p}_{si}", tag="sc")
            for h in range(2):
                nc.tensor.matmul(
                    scp[:, h, :],
                    kT[p][h * 64:(h + 1) * 64, si * 128:(si + 1) * 128],
                    qT[p][h * 64:(h + 1) * 64, t0:t0 + TB],
                    start=True, stop=True)
            ex = epool.tile([128, 2, TB], fr,
                            name=f"{R}ex_{tb}_{p}_{si}", tag="exp")
            nc.scalar.activation(ex[:], scp[:], Exp)
            for h in range(2):
                nc.tensor.matmul(
                    atp[h][:],
                    v_aug[:, si, p * 2 + h, :],
                    ex[:, h, :],
                    start=(si == 0), stop=(si == NS - 1),
                    skip_group_check=True)
        # normalize heads of this pair; bc reuses the freed at-slot
        for h in range(2):
            rec = npool.tile([1, TB], fr, name=f"{R}rc_{tb}_{p}_{h}",
                             tag="rec")
            with nc.allow_low_precision(reason="f32r recip"):
                nc.vector.reciprocal(rec[:], atp[h][HD:HD + 1, :])
            a2s = at2[p][h * 64:(h + 1) * 64, t0:t0 + TB]
            nc.vector.tensor_copy(a2s, atp[h][0:HD, :])
            bc = atps.tile([64, TB], f32, name=f"{R}bc_{tb}_{p}_{h}",
                           tag=f"at{h}")
            nc.tensor.matmul(bc[:], ones_r[:], rec[:], start=True, stop=True)
            nc.vector.tensor_tensor(a2s, a2s, bc[:],
                                    op=mybir.AluOpType.mult)

    def out_proj(tb):
        t0 = tb * TB
        for ts in range(TB // 128):
            tsl = t0 + ts * 128
            osb = opool.tile([128, D], f32, name=f"{R}osb_{tb}_{ts}",
                             tag="osb")
            for dc in range(2):
                ps = qkps.tile([128, 512], f32, name=f"{R}op_{tb}_{ts}_{dc}",
                               tag=f"qk{dc}")
                for p in range(2):
                    nc.tensor.matmul(
                        ps[:], at2[p][:, tsl:tsl + 128],
                        wo_t[:, p, dc * 512:(dc + 1) * 512],
                        start=(p == 0), stop=(p == 1))
                nc.vector.tensor_copy(osb[:, dc * 512:(dc + 1) * 512], ps[:])
            nc.sync.dma_start(out_d[tsl:tsl + 128, :], osb[:])

    qk_wave(wq_t, bq_t, qT, "q", 1, (0, 1))
    qk_wave(wq_t, bq_t, qT, "q", 1, (2, 3))
    attention_pair(0, 0)
    # k pair-1: low priority, fills PE idle under ACT during A(0,0)
    qk_wave(wk_t, bk_t, kT, "k", 1, (0, 1))
    qk_wave(wk_t, bk_t, kT, "k", 1, (2, 3))
    attention_pair(0, 1)
    for tb in range(1, NT):
        attention_pair(tb, 0)
        out_proj(tb - 1)
        attention_pair(tb, 1)
    out_proj(NT - 1)

    cctx.close()
    ictx.close()  # frees hT + qk psum
    octx.close()



def _get_program(reps=1):
    global _PROGRAM
    if _PROGRAM is None:
        _PROGRAM = {}
    if reps not in _PROGRAM:
        _PROGRAM[reps] = _build_program(reps)
    return _PROGRAM[reps]


def _shard_inputs(inputs):
    """Build the 8 per-core input maps from the full-problem inputs."""
    hs = np.asarray(inputs["hidden_states"], np.float32)
    pe = np.asarray(inputs["position_embeddings"], np.float32)
    Wq = np.asarray(inputs["Wq"], np.float32).reshape(D, H * HD)
    Wk = np.asarray(inputs["Wk"], np.float32).reshape(D, H * HD)
    Wv = np.asarray(inputs["Wv"], np.float32).reshape(D, H * HD)
    Wo = np.asarray(inputs["Wo"], np.float32)
    bq = np.asarray(inputs["bq"], np.float32).reshape(H * HD)
    bk = np.asarray(inputs["bk"], np.float32).reshape(H * HD)
    bv = np.asarray(inputs["bv"], np.float32).reshape(H * HD)

    xT = [np.ascontiguousarray(hs[b].T) for b in range(B)]
    pT = [np.ascontiguousarray(pe[b].T) for b in range(B)]

    in_maps = []
    for c in range(8):
        b, g = divmod(c, G)
        sel = slice(g * E, (g + 1) * E)
        in_maps.append({
            "xT": xT[b],
            "pT": pT[b],
            "wq": np.ascontiguousarray(Wq[:, sel]) * np.float32(SCALE),
            "wk": np.ascontiguousarray(Wk[:, sel]),
            "wv": np.ascontiguousarray(Wv[:, sel]),
            "wo": np.ascontiguousarray(Wo[sel, :]),
            "bq": (bq[sel] * np.float32(SCALE)).reshape(2, 128, 1).copy(),
            "bk": bk[sel].reshape(2, 128, 1).copy(),
            "bvr": np.tile(bv[sel][None, :], (128, 1)),
        })
    return in_maps


def _gather_outputs(results, inputs):
    bo = np.asarray(inputs["bo"], np.float32)
    out = np.empty((B, S, D), np.float32)
    for b in range(B):
        acc = results[4 * b]["out"].astype(np.float32).copy()
        for g in range(1, G):
            acc += results[4 * b + g]["out"]
        out[b] = acc + bo[None, :]
    return out


def kernel(**inputs):
    from concourse.bass_utils import run_bass_kernel_spmd

    nc = _get_program()
    in_maps = _shard_inputs(inputs)
    res = run_bass_kernel_spmd(nc, in_maps, list(range(8)))
    return _gather_outputs(res.results, inputs)



# revision 14
# speedup vs baseline: 1.4196x; 1.4196x over previous
"""Trainium2 Bass kernel for DFine multi-head attention (v2, bf16).

Problem: B=2, S=2048, D=1024, H=16 heads, HD=64.
Sharding over 8 cores: core c handles batch b=c//4 and head-group g=c%4
(4 heads). Each core computes its heads' attention and a partial
out-projection [2048, 1024]; the host sums the 4 partials per batch and
adds the output bias.

v2 design (vs fp32r baseline):
- All matmul operands bf16 (1 cyc/row at any moving size); psum f32.
- attnV swapped: stationary = exp-tile [128s x 128t], moving = v [128s, 65]
  (64 + ones column for the softmax denominator): 65-row matmuls instead of
  512-row ones -> halves attnV PE rows.
- attnV output lands [t, head_e] in psum, so the denominator is a
  per-partition scalar: reciprocal + tensor_scalar normalize, then a
  DMA transpose (xbar) produces the [e, t] layout for the out-projection.
- out-projection DMAs straight from PSUM to DRAM (no SBUF staging).
- h = x + pos precomputed on host; inputs DMAd bf16 (half the bytes).
- Static software pipeline: per si-step emit scores -> exp -> deferred
  attnV (one pair behind, so v/atp dependencies are off the critical
  path) -> projection/out_proj filler matmuls from a deadline queue.
"""

import sys
import numpy as np
import ml_dtypes

if "/opt/trn_rl_repo" not in sys.path:
    sys.path.insert(0, "/opt/trn_rl_repo")

B, S, D, H, HD = 2, 2048, 1024, 16, 64
G = 4          # heads per core
E = G * HD     # 256 per-core head width
T = S
KC = 8         # contraction chunks of 128 over D
TB = 512       # t-block
NT = T // TB   # 4
NS = T // 128  # 16 s-chunks
TCN = TB // 128  # 4 t-chunks per t-block
SCALE = HD ** -0.5

# pair order: all p=0 pairs first so kT/qT for p=1 and the second half of
# the projection work is not demanded in the first two pairs.
PAIRS = [(0, 0), (1, 0), (2, 0), (3, 0), (0, 1), (1, 1), (2, 1), (3, 1)]

_PROGRAM = None


def _build_program(reps=1):
    import concourse.bacc as bacc
    import concourse.tile as tile
    from concourse import mybir

    f32 = mybir.dt.float32
    bf16 = mybir.dt.bfloat16

    nc = bacc.Bacc("TRN2", target_bir_lowering=False, debug=False)

    hT_d = nc.declare_dram_parameter("hT", [D, T], bf16, isOutput=False)
    xT_d = nc.declare_dram_parameter("xT", [D, T], bf16, isOutput=False)
    wq_d = nc.declare_dram_parameter("wq", [D, E], bf16, isOutput=False)
    wk_d = nc.declare_dram_parameter("wk", [D, E], bf16, isOutput=False)
    wv_d = nc.declare_dram_parameter("wv", [D, E], bf16, isOutput=False)
    wo_d = nc.declare_dram_parameter("wo", [E, D], bf16, isOutput=False)
    bq_d = nc.declare_dram_parameter("bq", [2, 128, 1], f32, isOutput=False)
    bk_d = nc.declare_dram_parameter("bk", [2, 128, 1], f32, isOutput=False)
    bv_d = nc.declare_dram_parameter("bvr", [128, E], f32, isOutput=False)
    out_d = nc.declare_dram_parameter("out", [T, D], f32, isOutput=True)

    with tile.TileContext(nc) as tc:
        for rep in range(reps):
            _build_body(nc, tc, mybir, rep,
                        (hT_d, xT_d, wq_d, wk_d, wv_d, wo_d, bq_d, bk_d,
                         bv_d, out_d))

    nc.compile()
    return nc


def _build_body(nc, tc, mybir, rep, drams):
    from contextlib import ExitStack

    f32 = mybir.dt.float32
    bf16 = mybir.dt.bfloat16
    Exp = mybir.ActivationFunctionType.Exp
    Add = mybir.AluOpType.add
    Mult = mybir.AluOpType.mult
    (hT_d, xT_d, wq_d, wk_d, wv_d, wo_d, bq_d, bk_d, bv_d, out_d) = drams
    R = f"r{rep}_"

    octx = ExitStack()
    wpool = octx.enter_context(tc.tile_pool(name=f"{R}wpool", bufs=1))
    expool = octx.enter_context(tc.tile_pool(name=f"{R}expool", bufs=18))
    a2pool = octx.enter_context(tc.tile_pool(name=f"{R}a2pool", bufs=2))
    ospool = octx.enter_context(tc.tile_pool(name=f"{R}ospool", bufs=4))
    scps = octx.enter_context(tc.tile_pool(name=f"{R}scps", bufs=2,
                                           space="PSUM"))
    atps = octx.enter_context(tc.tile_pool(name=f"{R}atps", bufs=1,
                                           space="PSUM"))
    opps = octx.enter_context(tc.tile_pool(name=f"{R}opps", bufs=1,
                                           space="PSUM"))

    # ---- persistent SBUF tiles ----
    wq_t = wpool.tile([128, KC, E], bf16, name=f"{R}wq_t")
    wk_t = wpool.tile([128, KC, E], bf16, name=f"{R}wk_t")
    wv_t = wpool.tile([128, KC, E], bf16, name=f"{R}wv_t")
    wo_t = wpool.tile([128, 2, D], bf16, name=f"{R}wo_t")
    bq_t = wpool.tile([128, 2, 1], f32, name=f"{R}bq_t")
    bk_t = wpool.tile([128, 2, 1], f32, name=f"{R}bk_t")
    bv_t = wpool.tile([128, E], f32, name=f"{R}bv_t")
    hT_t = wpool.tile([128, KC, T], bf16, name=f"{R}hT_t")
    xT_t = wpool.tile([128, KC, T], bf16, name=f"{R}xT_t")
    qT = [wpool.tile([128, T], bf16, name=f"{R}qT{p}") for p in range(2)]
    kT = [wpool.tile([128, T], bf16, name=f"{R}kT{p}") for p in range(2)]
    v_aug = wpool.tile([128, NS, G, 66], bf16, name=f"{R}v_aug")
    at2 = [wpool.tile([128, T], bf16, name=f"{R}at2_{p}") for p in range(2)]
    rec8 = wpool.tile([128, 8, 1], f32, name=f"{R}rec8")
    onecol = wpool.tile([128, NS, G, 1], bf16, name=f"{R}onecol")

    nc.gpsimd.memset(onecol[:], 1.0)
    nc.gpsimd.tensor_copy(v_aug[:, :, :, 64:65], onecol[:])

    # ---- DMA emission (SP queue, FIFO) ----
    # wk, wq first; then hT t-block 0 chunk-by-chunk with the first k/q
    # projection matmuls chasing each chunk so scores can start ~9us in.
    nc.sync.dma_start(
        wk_t[:], wk_d[:].rearrange("(c p) e -> p c e", p=128))
    nc.sync.dma_start(
        wq_t[:], wq_d[:].rearrange("(c p) e -> p c e", p=128))

    ps_k0 = opps.tile([128, TB], f32, name=f"{R}k0s0ps", tag="op0")
    ps_q0 = opps.tile([128, TB], f32, name=f"{R}q0t0ps", tag="op1")
    for k in range(KC):
        nc.sync.dma_start(hT_t[:, k, 0:TB], hT_d[k * 128:(k + 1) * 128, 0:TB])
        nc.tensor.matmul(ps_k0[:], wk_t[:, k, 0:128], hT_t[:, k, 0:TB],
                         start=(k == 0), stop=(k == KC - 1))
        nc.tensor.matmul(ps_q0[:], wq_t[:, k, 0:128], hT_t[:, k, 0:TB],
                         start=(k == 0), stop=(k == KC - 1))
    # biases land before the first drains need them
    nc.sync.dma_start(bk_t[:], bk_d[:].rearrange("c p o -> p c o"))
    nc.sync.dma_start(bq_t[:], bq_d[:].rearrange("c p o -> p c o"))
    nc.sync.dma_start(bv_t[:], bv_d[:])
    nc.vector.tensor_scalar(kT[0][:, 0:TB], ps_k0[:], bk_t[:, 0, :], None,
                            Add)
    nc.vector.tensor_scalar(qT[0][:, 0:TB], ps_q0[:], bq_t[:, 0, :], None,
                            Add)

    for qd in range(1, 4):
        nc.sync.dma_start(
            hT_t[:, :, qd * TB:(qd + 1) * TB],
            hT_d[:, qd * TB:(qd + 1) * TB].rearrange("(c p) t -> p c t",
                                                     p=128))
    nc.sync.dma_start(
        wv_t[:], wv_d[:].rearrange("(c p) e -> p c e", p=128))
    for qd in range(4):
        nc.sync.dma_start(
            xT_t[:, :, qd * TB:(qd + 1) * TB],
            xT_d[:, qd * TB:(qd + 1) * TB].rearrange("(c p) t -> p c t",
                                                     p=128))
    nc.sync.dma_start(
        wo_t[:], wo_d[:].rearrange("(c p) d -> p c d", p=128))

    # ---- filler queue: deadline-ordered projection / out_proj work ----
    tag_i = [0]

    def next_tag():
        t = f"op{tag_i[0] % 2}"
        tag_i[0] += 1
        return t

    def qk_group(w_t, b_t, dstT, p, blk, nm):
        box = {}
        tag = [None]

        def mk_mm(k):
            def f():
                if k == 0:
                    tag[0] = next_tag()
                    box["ps"] = opps.tile([128, TB], f32,
                                          name=f"{R}{nm}ps", tag=tag[0])
                nc.tensor.matmul(box["ps"][:],
                                 w_t[:, k, p * 128:(p + 1) * 128],
                                 hT_t[:, k, blk * TB:(blk + 1) * TB],
                                 start=(k == 0), stop=(k == KC - 1))
            return f

        ops = [(213, mk_mm(k)) for k in range(KC)]

        def drain():
            nc.vector.tensor_scalar(dstT[p][:, blk * TB:(blk + 1) * TB],
                                    box["ps"][:], b_t[:, p, :], None, Add)
        ops.append((0, drain))
        return ops

    def v_group(j):
        # si pair (2j, 2j+1): two 8-matmul chains into one psum bank
        box = {}
        tag = [None]

        def mk_mm(k, jj):
            def f():
                if k == 0 and jj == 0:
                    tag[0] = next_tag()
                    box["ps"] = opps.tile([128, 2, E], f32,
                                          name=f"{R}v{j}ps", tag=tag[0])
                si = 2 * j + jj
                # HW: start=True zeroes the whole psum bank, so only the
                # first chain in the bank starts; the sibling accumulates.
                nc.tensor.matmul(box["ps"][:, jj, :],
                                 xT_t[:, k, si * 128:(si + 1) * 128],
                                 wv_t[:, k, :],
                                 start=(k == 0 and jj == 0),
                                 stop=(k == KC - 1),
                                 skip_group_check=True)
            return f

        ops = []
        for k in range(KC):
            for jj in range(2):
                ops.append((107, mk_mm(k, jj)))

        def mk_drain(jj):
            def f():
                si = 2 * j + jj
                nc.vector.tensor_tensor(
                    v_aug[:, si, :, 0:64],
                    box["ps"][:, jj, :].rearrange("p (g e) -> p g e", g=G),
                    bv_t[:].rearrange("p (g e) -> p g e", g=G),
                    op=Add)
            return f
        ops.append((0, mk_drain(0)))
        ops.append((0, mk_drain(1)))
        return ops

    def out_group(tb, ts, dc):
        box = {}
        tag = [None]

        def mk_mm(p):
            def f():
                if p == 0:
                    tag[0] = next_tag()
                    box["ps"] = opps.tile([128, TB], f32,
                                          name=f"{R}o{tb}_{ts}_{dc}ps",
                                          tag=tag[0])
                nc.tensor.matmul(box["ps"][:],
                                 at2[p][:, tb * TB + ts * 128:
                                        tb * TB + ts * 128 + 128],
                                 wo_t[:, p, dc * TB:(dc + 1) * TB],
                                 start=(p == 0), stop=(p == 1))
            return f

        def drain():
            box["osb"] = ospool.tile([128, TB], f32,
                                     name=f"{R}o{tb}_{ts}_{dc}sb", tag="os")
            nc.vector.tensor_copy(box["osb"][:], box["ps"][:])

        def dma():
            nc.sync.dma_start(
                out_d[tb * TB + ts * 128: tb * TB + (ts + 1) * 128,
                      dc * TB:(dc + 1) * TB],
                box["osb"][:])
        return [(213, mk_mm(0)), (213, mk_mm(1)), (0, drain), (0, dma)]

    fillers = []
    filler_by_key = {}

    def filler_item(ready, ops, key=None):
        it = {"ready": ready, "ops": ops, "i": 0, "key": key}
        fillers.append(it)
        if key is not None:
            filler_by_key[key] = it

    def ensure(key):
        """Force-emit every remaining op of the filler item `key` so a
        consumer emitted next observes its writes (tile deps only order
        instructions that are already emitted)."""
        it = filler_by_key.get(key)
        if it is None:
            return
        while it["i"] < len(it["ops"]):
            _, fn = it["ops"][it["i"]]
            it["i"] += 1
            fn()

    # deadline-ordered: (the scan picks the first *ready* item)
    filler_item(1, qk_group(wk_t, bk_t, kT, 0, 1, "k0s1"), ("k", 0, 1))
    filler_item(4, qk_group(wk_t, bk_t, kT, 0, 2, "k0s2"), ("k", 0, 2))
    filler_item(7, qk_group(wk_t, bk_t, kT, 0, 3, "k0s3"), ("k", 0, 3))
    filler_item(2, qk_group(wq_t, bq_t, qT, 0, 1, "q0t1"), ("q", 0, 1))
    filler_item(11, v_group(0), ("v", 0))
    filler_item(11, v_group(1), ("v", 1))
    filler_item(14, v_group(2), ("v", 2))
    filler_item(14, v_group(3), ("v", 3))
    filler_item(4, qk_group(wq_t, bq_t, qT, 0, 2, "q0t2"), ("q", 0, 2))
    filler_item(17, v_group(4), ("v", 4))
    filler_item(17, v_group(5), ("v", 5))
    filler_item(20, v_group(6), ("v", 6))
    filler_item(20, v_group(7), ("v", 7))
    filler_item(7, qk_group(wq_t, bq_t, qT, 0, 3, "q0t3"), ("q", 0, 3))
    filler_item(9, qk_group(wq_t, bq_t, qT, 1, 0, "q1t0"), ("q", 1, 0))
    filler_item(9, qk_group(wk_t, bk_t, kT, 1, 0, "k1s0"), ("k", 1, 0))
    filler_item(9, qk_group(wk_t, bk_t, kT, 1, 1, "k1s1"), ("k", 1, 1))
    filler_item(9, qk_group(wk_t, bk_t, kT, 1, 2, "k1s2"), ("k", 1, 2))
    filler_item(9, qk_group(wk_t, bk_t, kT, 1, 3, "k1s3"), ("k", 1, 3))
    filler_item(9, qk_group(wq_t, bq_t, qT, 1, 1, "q1t1"), ("q", 1, 1))
    filler_item(9, qk_group(wq_t, bq_t, qT, 1, 2, "q1t2"), ("q", 1, 2))
    filler_item(9, qk_group(wq_t, bq_t, qT, 1, 3, "q1t3"), ("q", 1, 3))

    def run_fillers(step, budget):
        spent = 0
        while spent < budget:
            it = None
            for x in fillers:
                if x["i"] < len(x["ops"]) and x["ready"] <= step:
                    it = x
                    break
            if it is None:
                return
            cost, fn = it["ops"][it["i"]]
            it["i"] += 1
            fn()
            spent += cost

    # ---- attention machinery ----
    ex_store = {}
    atp_store = {}
    a2n_store = {}

    def scores_exp(pi, si):
        tb, p = PAIRS[pi]
        ensure(("k", p, si // 4))
        ensure(("q", p, tb))
        scp = scps.tile([128, 2, TB], f32, name=f"{R}sc{pi}_{si}", tag="sc")
        for h in range(2):
            nc.tensor.matmul(
                scp[:, h, :],
                kT[p][h * 64:(h + 1) * 64, si * 128:(si + 1) * 128],
                qT[p][h * 64:(h + 1) * 64, tb * TB:(tb + 1) * TB],
                start=True, stop=True)
        ex = expool.tile([128, 2, TB], bf16, name=f"{R}ex{pi}_{si}", tag="ex")
        nc.scalar.activation(ex[:], scp[:], Exp)
        ex_store[(pi, si)] = ex

    def attnv(pi, si):
        tb, p = PAIRS[pi]
        ensure(("v", si // 2))
        if si == 0:
            atp_store[pi] = atps.tile([128, 8, 128], f32,
                                      name=f"{R}atp{pi}", tag="at")
        atp = atp_store[pi]
        ex = ex_store.pop((pi, si))
        for h in range(2):
            for tcn in range(TCN):
                u = h * TCN + tcn
                # start=True zeroes the whole bank on HW: chains u=0..3
                # live in bank A (zeroed by u==0), u=4..7 in bank B
                # (zeroed by u==4); all siblings accumulate.
                nc.tensor.matmul(
                    atp[:, u, 0:65],
                    ex[:, h, tcn * 128:(tcn + 1) * 128],
                    v_aug[:, si, p * 2 + h, 0:65],
                    start=(si == 0 and u % 4 == 0), stop=(si == NS - 1),
                    skip_group_check=True)

    def normalize(pi):
        tb, p = PAIRS[pi]
        atp = atp_store.pop(pi)
        a2n = a2pool.tile([128, TCN, 128], bf16, name=f"{R}a2n{pi}",
                          tag="a2n")
        nc.vector.reciprocal(rec8[:], atp[:, :, 64:65])
        for u in range(8):
            h, tcn = divmod(u, TCN)
            nc.vector.tensor_scalar(a2n[:, tcn, h * 64:(h + 1) * 64],
                              atp[:, u, 0:64], rec8[:, u, :], None, Mult)
        a2n_store[pi] = a2n

    def dma_transpose(pi):
        tb, p = PAIRS[pi]
        a2n = a2n_store.pop(pi)
        for tcn in range(TCN):
            nc.sync.dma_start_transpose(
                at2[p][:, tb * TB + tcn * 128: tb * TB + (tcn + 1) * 128],
                a2n[:, tcn, :])

    # ---- main pipeline ----
    for pi in range(len(PAIRS)):
        for si in range(NS):
            gs = pi * NS + si
            scores_exp(pi, si)
            if pi == len(PAIRS) - 1:
                # last pair: drain the previous pair's attnV at double
                # rate, normalize it mid-pair, then chase this pair's own
                # attnV so the tail is short.
                if si < 8:
                    attnv(pi - 1, 2 * si)
                    attnv(pi - 1, 2 * si + 1)
                    budget = 220
                elif si == 8:
                    normalize(pi - 1)
                    dma_transpose(pi - 1)
                    # out_proj for the second-to-last tb becomes filler work
                    tb_p = PAIRS[pi - 1][0]
                    for ts in range(TCN):
                        for dc in range(2):
                            filler_item(gs + 2 + (ts * 2 + dc),
                                        out_group(tb_p, ts, dc))
                    budget = 430
                else:  # si 9..15: emit attnV(pi) for si 0..2*(si-9)+1
                    attnv(pi, 2 * (si - 9))
                    attnv(pi, 2 * (si - 9) + 1)
                    budget = 220
            elif pi > 0:
                attnv(pi - 1, si)
                budget = 430
            else:
                budget = 650
            run_fillers(gs, budget)
        if 0 < pi < len(PAIRS) - 1:
            normalize(pi - 1)
            dma_transpose(pi - 1)
            if PAIRS[pi - 1][1] == 1:
                # at2 for this tb is now complete on both pairs ->
                # out-projection becomes available filler work
                tb = PAIRS[pi - 1][0]
                for ts in range(TCN):
                    for dc in range(2):
                        filler_item(pi * NS + 3 + 2 * (ts * 2 + dc),
                                    out_group(tb, ts, dc))

    # ---- tail: finish last pair, then its out_proj ----
    for si in range(14, NS):
        attnv(len(PAIRS) - 1, si)
    normalize(len(PAIRS) - 1)
    dma_transpose(len(PAIRS) - 1)
    # remaining fillers (any stragglers, incl. leftover out_proj groups)
    run_fillers(10 ** 9, 10 ** 9)
    # out_proj for the last tb over 4 psum banks (scores banks are free now)
    tb = PAIRS[-1][0]
    for ts in range(TCN):
        for dc in range(2):
            if (ts * 2 + dc) % 2 == 0:
                ps = opps.tile([128, TB], f32, name=f"{R}ot{ts}_{dc}",
                               tag=next_tag())
            else:
                ps = scps.tile([128, TB], f32, name=f"{R}ot{ts}_{dc}",
                               tag="sc")
            for p in range(2):
                nc.tensor.matmul(
                    ps[:],
                    at2[p][:, tb * TB + ts * 128:tb * TB + ts * 128 + 128],
                    wo_t[:, p, dc * TB:(dc + 1) * TB],
                    start=(p == 0), stop=(p == 1))
            osb = ospool.tile([128, TB], f32, name=f"{R}ot{ts}_{dc}sb",
                              tag="os")
            nc.vector.tensor_copy(osb[:], ps[:])
            nc.sync.dma_start(
                out_d[tb * TB + ts * 128: tb * TB + (ts + 1) * 128,
                      dc * TB:(dc + 1) * TB],
                osb[:])

    octx.close()


def _get_program(reps=1):
    global _PROGRAM
    if _PROGRAM is None:
        _PROGRAM = {}
    if reps not in _PROGRAM:
        _PROGRAM[reps] = _build_program(reps)
    return _PROGRAM[reps]


def _shard_inputs(inputs):
    """Build the 8 per-core input maps from the full-problem inputs."""
    bf16 = ml_dtypes.bfloat16
    hs = np.asarray(inputs["hidden_states"], np.float32)
    pe = np.asarray(inputs["position_embeddings"], np.float32)
    Wq = np.asarray(inputs["Wq"], np.float32).reshape(D, H * HD)
    Wk = np.asarray(inputs["Wk"], np.float32).reshape(D, H * HD)
    Wv = np.asarray(inputs["Wv"], np.float32).reshape(D, H * HD)
    Wo = np.asarray(inputs["Wo"], np.float32)
    bq = np.asarray(inputs["bq"], np.float32).reshape(H * HD)
    bk = np.asarray(inputs["bk"], np.float32).reshape(H * HD)
    bv = np.asarray(inputs["bv"], np.float32).reshape(H * HD)

    h = hs + pe
    hT = [np.ascontiguousarray(h[b].T).astype(bf16) for b in range(B)]
    xT = [np.ascontiguousarray(hs[b].T).astype(bf16) for b in range(B)]

    in_maps = []
    for c in range(8):
        b, g = divmod(c, G)
        sel = slice(g * E, (g + 1) * E)
        in_maps.append({
            "hT": hT[b],
            "xT": xT[b],
            "wq": (np.ascontiguousarray(Wq[:, sel])
                   * np.float32(SCALE)).astype(bf16),
            "wk": np.ascontiguousarray(Wk[:, sel]).astype(bf16),
            "wv": np.ascontiguousarray(Wv[:, sel]).astype(bf16),
            "wo": np.ascontiguousarray(Wo[sel, :]).astype(bf16),
            "bq": (bq[sel] * np.float32(SCALE)).reshape(2, 128, 1).copy(),
            "bk": bk[sel].reshape(2, 128, 1).copy(),
            "bvr": np.tile(bv[sel][None, :], (128, 1)),
        })
    return in_maps


def _gather_outputs(results, inputs):
    bo = np.asarray(inputs["bo"], np.float32)
    out = np.empty((B, S, D), np.float32)
    for b in range(B):
        acc = results[4 * b]["out"].astype(np.float32).copy()
        for g in range(1, G):
            acc += results[4 * b + g]["out"]
        out[b] = acc + bo[None, :]
    return out


def kernel(**inputs):
    from concourse.bass_utils import run_bass_kernel_spmd

    nc = _get_program()
    in_maps = _shard_inputs(inputs)
    res = run_bass_kernel_spmd(nc, in_maps, list(range(8)))
    return _gather_outputs(res.results, inputs)


# revision 25
# speedup vs baseline: 1.4773x; 1.0407x over previous
"""Trainium2 Bass kernel for DFine multi-head attention (v2, bf16).

Problem: B=2, S=2048, D=1024, H=16 heads, HD=64.
Sharding over 8 cores: core c handles batch b=c//4 and head-group g=c%4
(4 heads). Each core computes its heads' attention and a partial
out-projection [2048, 1024]; the host sums the 4 partials per batch and
adds the output bias.

v2 design (vs fp32r baseline):
- All matmul operands bf16 (1 cyc/row at any moving size); psum f32.
- attnV swapped: stationary = exp-tile [128s x 128t], moving = v [128s, 65]
  (64 + ones column for the softmax denominator): 65-row matmuls instead of
  512-row ones -> halves attnV PE rows.
- attnV output lands [t, head_e] in psum, so the denominator is a
  per-partition scalar: reciprocal + tensor_scalar normalize, then a
  DMA transpose (xbar) produces the [e, t] layout for the out-projection.
- out-projection DMAs straight from PSUM to DRAM (no SBUF staging).
- h = x + pos precomputed on host; inputs DMAd bf16 (half the bytes).
- Static software pipeline: per si-step emit scores -> exp -> deferred
  attnV (one pair behind, so v/atp dependencies are off the critical
  path) -> projection/out_proj filler matmuls from a deadline queue.
"""

import sys
import numpy as np
import ml_dtypes

if "/opt/trn_rl_repo" not in sys.path:
    sys.path.insert(0, "/opt/trn_rl_repo")

B, S, D, H, HD = 2, 2048, 1024, 16, 64
G = 4          # heads per core
E = G * HD     # 256 per-core head width
T = S
KC = 8         # contraction chunks of 128 over D
TB = 512       # t-block
NT = T // TB   # 4
NS = T // 128  # 16 s-chunks
TCN = TB // 128  # 4 t-chunks per t-block
SCALE = HD ** -0.5

# pair order: all p=0 pairs first so kT/qT for p=1 and the second half of
# the projection work is not demanded in the first two pairs.
PAIRS = [(0, 0), (1, 0), (2, 0), (3, 0), (0, 1), (1, 1), (2, 1), (3, 1)]

_PROGRAM = None
_DBG = False


def _build_program(reps=1):
    import concourse.bacc as bacc
    import concourse.tile as tile
    from concourse import mybir

    f32 = mybir.dt.float32
    bf16 = mybir.dt.bfloat16

    nc = bacc.Bacc("TRN2", target_bir_lowering=False, debug=False)

    hT_d = nc.declare_dram_parameter("hT", [D, T], bf16, isOutput=False)
    xT_d = nc.declare_dram_parameter("xT", [D, T], bf16, isOutput=False)
    wq_d = nc.declare_dram_parameter("wq", [D, E], bf16, isOutput=False)
    wk_d = nc.declare_dram_parameter("wk", [D, E], bf16, isOutput=False)
    wv_d = nc.declare_dram_parameter("wv", [D, E], bf16, isOutput=False)
    wo_d = nc.declare_dram_parameter("wo", [E, D], bf16, isOutput=False)
    bq_d = nc.declare_dram_parameter("bq", [2, 128, 1], f32, isOutput=False)
    bk_d = nc.declare_dram_parameter("bk", [2, 128, 1], f32, isOutput=False)
    bv_d = nc.declare_dram_parameter("bvr", [128, E], f32, isOutput=False)
    out_d = nc.declare_dram_parameter("out", [T, D], bf16, isOutput=True)

    with tile.TileContext(nc) as tc:
        for rep in range(reps):
            _build_body(nc, tc, mybir, rep,
                        (hT_d, xT_d, wq_d, wk_d, wv_d, wo_d, bq_d, bk_d,
                         bv_d, out_d))

    nc.compile()
    return nc


def _build_body(nc, tc, mybir, rep, drams):
    from contextlib import ExitStack

    f32 = mybir.dt.float32
    bf16 = mybir.dt.bfloat16
    Exp = mybir.ActivationFunctionType.Exp
    Add = mybir.AluOpType.add
    Mult = mybir.AluOpType.mult
    (hT_d, xT_d, wq_d, wk_d, wv_d, wo_d, bq_d, bk_d, bv_d, out_d) = drams
    R = f"r{rep}_"

    octx = ExitStack()
    wpool = octx.enter_context(tc.tile_pool(name=f"{R}wpool", bufs=1))
    expool = octx.enter_context(tc.tile_pool(name=f"{R}expool", bufs=18))
    a2pool = octx.enter_context(tc.tile_pool(name=f"{R}a2pool", bufs=2))
    ospool = octx.enter_context(tc.tile_pool(name=f"{R}ospool", bufs=4))
    scps = octx.enter_context(tc.tile_pool(name=f"{R}scps", bufs=2,
                                           space="PSUM"))
    atps = octx.enter_context(tc.tile_pool(name=f"{R}atps", bufs=1,
                                           space="PSUM"))
    opps = octx.enter_context(tc.tile_pool(name=f"{R}opps", bufs=1,
                                           space="PSUM"))

    # ---- persistent SBUF tiles ----
    wq_t = wpool.tile([128, KC, E], bf16, name=f"{R}wq_t")
    wk_t = wpool.tile([128, KC, E], bf16, name=f"{R}wk_t")
    wv_t = wpool.tile([128, KC, E], bf16, name=f"{R}wv_t")
    wo_t = wpool.tile([128, 2, D], bf16, name=f"{R}wo_t")
    bq_t = wpool.tile([128, 2, 1], f32, name=f"{R}bq_t")
    bk_t = wpool.tile([128, 2, 1], f32, name=f"{R}bk_t")
    bv_t = wpool.tile([128, E], f32, name=f"{R}bv_t")
    hT_t = wpool.tile([128, KC, T], bf16, name=f"{R}hT_t")
    xT_t = wpool.tile([128, KC, T], bf16, name=f"{R}xT_t")
    qT = [wpool.tile([128, T], bf16, name=f"{R}qT{p}") for p in range(2)]
    kT = [wpool.tile([128, T], bf16, name=f"{R}kT{p}") for p in range(2)]
    v_aug = wpool.tile([128, NS, G, 66], bf16, name=f"{R}v_aug")
    at2 = [wpool.tile([128, T], bf16, name=f"{R}at2_{p}") for p in range(2)]
    rec8 = wpool.tile([128, 8, 1], f32, name=f"{R}rec8")
    onecol = wpool.tile([128, NS, G, 1], bf16, name=f"{R}onecol")
    id_t = wpool.tile([128, 128], bf16, name=f"{R}id_t")

    nc.gpsimd.memset(onecol[:], 1.0)
    nc.gpsimd.tensor_copy(v_aug[:, :, :, 64:65], onecol[:])
    from concourse import masks
    masks.make_identity(nc, id_t[:])

    # ---- DMA emission (SP queue, FIFO) ----
    # wk, wq first; then hT t-block 0 chunk-by-chunk with the first k/q
    # projection matmuls chasing each chunk so scores can start ~9us in.
    nc.sync.dma_start(
        wk_t[:], wk_d[:].rearrange("(c p) e -> p c e", p=128))
    nc.sync.dma_start(
        wq_t[:], wq_d[:].rearrange("(c p) e -> p c e", p=128))
    # tiny bias DMAs issue early (HWDGE issue slots serialize at ~650ns
    # each, so putting them after the 8 hT chunks would delay the first
    # q/k drains by ~3us)
    nc.sync.dma_start(bk_t[:], bk_d[:].rearrange("c p o -> p c o"))
    nc.sync.dma_start(bq_t[:], bq_d[:].rearrange("c p o -> p c o"))
    nc.sync.dma_start(bv_t[:], bv_d[:])

    ps_k0 = opps.tile([128, TB], f32, name=f"{R}k0s0ps", tag="op0")
    ps_q0 = opps.tile([128, TB], f32, name=f"{R}q0t0ps", tag="op1")
    for k in range(KC):
        nc.sync.dma_start(hT_t[:, k, 0:TB], hT_d[k * 128:(k + 1) * 128, 0:TB])
        nc.tensor.matmul(ps_k0[:], wk_t[:, k, 0:128], hT_t[:, k, 0:TB],
                         start=(k == 0), stop=(k == KC - 1))
        nc.tensor.matmul(ps_q0[:], wq_t[:, k, 0:128], hT_t[:, k, 0:TB],
                         start=(k == 0), stop=(k == KC - 1))
    nc.vector.tensor_scalar(kT[0][:, 0:TB], ps_k0[:], bk_t[:, 0, :], None,
                            Add)
    nc.vector.tensor_scalar(qT[0][:, 0:TB], ps_q0[:], bq_t[:, 0, :], None,
                            Add)

    for qd in range(1, 4):
        nc.sync.dma_start(
            hT_t[:, :, qd * TB:(qd + 1) * TB],
            hT_d[:, qd * TB:(qd + 1) * TB].rearrange("(c p) t -> p c t",
                                                     p=128))
    nc.sync.dma_start(
        wv_t[:], wv_d[:].rearrange("(c p) e -> p c e", p=128))
    for qd in range(4):
        nc.sync.dma_start(
            xT_t[:, :, qd * TB:(qd + 1) * TB],
            xT_d[:, qd * TB:(qd + 1) * TB].rearrange("(c p) t -> p c t",
                                                     p=128))
    nc.sync.dma_start(
        wo_t[:], wo_d[:].rearrange("(c p) d -> p c d", p=128))

    # ---- filler queue: deadline-ordered projection / out_proj work ----
    tag_i = [0]

    def next_tag():
        t = f"op{tag_i[0] % 2}"
        tag_i[0] += 1
        return t

    def qk_group(w_t, b_t, dstT, p, blk, nm):
        box = {}
        tag = [None]

        def mk_mm(k):
            def f():
                if k == 0:
                    tag[0] = next_tag()
                    box["ps"] = opps.tile([128, TB], f32,
                                          name=f"{R}{nm}ps", tag=tag[0])
                nc.tensor.matmul(box["ps"][:],
                                 w_t[:, k, p * 128:(p + 1) * 128],
                                 hT_t[:, k, blk * TB:(blk + 1) * TB],
                                 start=(k == 0), stop=(k == KC - 1))
            return f

        ops = [(213, mk_mm(k)) for k in range(KC)]

        def drain():
            nc.vector.tensor_scalar(dstT[p][:, blk * TB:(blk + 1) * TB],
                                    box["ps"][:], b_t[:, p, :], None, Add)
        ops.append((0, drain))
        return ops

    def v_group(j):
        # si pair (2j, 2j+1): two 8-matmul chains into one psum bank
        box = {}
        tag = [None]

        def mk_mm(k, jj):
            def f():
                if k == 0 and jj == 0:
                    tag[0] = next_tag()
                    box["ps"] = opps.tile([128, 2, E], f32,
                                          name=f"{R}v{j}ps", tag=tag[0])
                si = 2 * j + jj
                # HW: start=True zeroes the whole psum bank, so only the
                # first chain in the bank starts; the sibling accumulates.
                nc.tensor.matmul(box["ps"][:, jj, :],
                                 xT_t[:, k, si * 128:(si + 1) * 128],
                                 wv_t[:, k, :],
                                 start=(k == 0 and jj == 0),
                                 stop=(k == KC - 1),
                                 skip_group_check=True)
            return f

        ops = []
        for k in range(KC):
            for jj in range(2):
                ops.append((107, mk_mm(k, jj)))

        def mk_drain(jj):
            def f():
                si = 2 * j + jj
                nc.vector.tensor_tensor(
                    v_aug[:, si, :, 0:64],
                    box["ps"][:, jj, :].rearrange("p (g e) -> p g e", g=G),
                    bv_t[:].rearrange("p (g e) -> p g e", g=G),
                    op=Add)
            return f
        ops.append((0, mk_drain(0)))
        ops.append((0, mk_drain(1)))
        return ops

    def out_unit(tb, ts):
        # one 128-row output slice: dc0 fills one psum bank, dc1 the
        # other; both drain into one bf16 staging row -> single DMA.
        box = {}

        def mk_mm(dc, p):
            def f():
                if p == 0:
                    box[dc] = opps.tile([128, TB], f32,
                                        name=f"{R}o{tb}_{ts}_{dc}ps",
                                        tag=next_tag())
                nc.tensor.matmul(box[dc][:],
                                 at2[p][:, tb * TB + ts * 128:
                                        tb * TB + ts * 128 + 128],
                                 wo_t[:, p, dc * TB:(dc + 1) * TB],
                                 start=(p == 0), stop=(p == 1))
            return f

        def mk_drain(dc):
            def f():
                if dc == 0:
                    box["osb"] = ospool.tile([128, D], bf16,
                                             name=f"{R}o{tb}_{ts}sb",
                                             tag="os")
                nc.vector.tensor_copy(
                    box["osb"][:, dc * TB:(dc + 1) * TB], box[dc][:])
            return f

        def dma():
            nc.sync.dma_start(
                out_d[tb * TB + ts * 128: tb * TB + (ts + 1) * 128, :],
                box["osb"][:])
        return [(213, mk_mm(0, 0)), (213, mk_mm(0, 1)), (0, mk_drain(0)),
                (213, mk_mm(1, 0)), (213, mk_mm(1, 1)), (0, mk_drain(1)),
                (0, dma)]

    fillers = []
    filler_by_key = {}

    def filler_item(ready, ops, key=None):
        it = {"ready": ready, "ops": ops, "i": 0, "key": key}
        fillers.append(it)
        if key is not None:
            filler_by_key[key] = it

    def ensure(key, _dbg=[0]):
        """Force-emit every remaining op of the filler item `key` so a
        consumer emitted next observes its writes (tile deps only order
        instructions that are already emitted)."""
        it = filler_by_key.get(key)
        if it is None:
            return
        n = len(it["ops"]) - it["i"]
        if n > 0 and _DBG:
            print(f"ENSURE {key} forces {n} ops at step {CUR_STEP[0]}")
        while it["i"] < len(it["ops"]):
            _, fn = it["ops"][it["i"]]
            it["i"] += 1
            fn()

    # deadline-ordered: (the scan picks the first *ready* item)
    filler_item(1, qk_group(wk_t, bk_t, kT, 0, 1, "k0s1"), ("k", 0, 1))
    filler_item(4, qk_group(wk_t, bk_t, kT, 0, 2, "k0s2"), ("k", 0, 2))
    filler_item(7, qk_group(wk_t, bk_t, kT, 0, 3, "k0s3"), ("k", 0, 3))
    filler_item(2, qk_group(wq_t, bq_t, qT, 0, 1, "q0t1"), ("q", 0, 1))
    filler_item(11, v_group(0), ("v", 0))
    filler_item(11, v_group(1), ("v", 1))
    filler_item(14, v_group(2), ("v", 2))
    filler_item(14, v_group(3), ("v", 3))
    filler_item(4, qk_group(wq_t, bq_t, qT, 0, 2, "q0t2"), ("q", 0, 2))
    filler_item(17, v_group(4), ("v", 4))
    filler_item(17, v_group(5), ("v", 5))
    filler_item(20, v_group(6), ("v", 6))
    filler_item(20, v_group(7), ("v", 7))
    filler_item(7, qk_group(wq_t, bq_t, qT, 0, 3, "q0t3"), ("q", 0, 3))
    filler_item(9, qk_group(wq_t, bq_t, qT, 1, 0, "q1t0"), ("q", 1, 0))
    filler_item(9, qk_group(wk_t, bk_t, kT, 1, 0, "k1s0"), ("k", 1, 0))
    filler_item(9, qk_group(wk_t, bk_t, kT, 1, 1, "k1s1"), ("k", 1, 1))
    filler_item(9, qk_group(wk_t, bk_t, kT, 1, 2, "k1s2"), ("k", 1, 2))
    filler_item(9, qk_group(wk_t, bk_t, kT, 1, 3, "k1s3"), ("k", 1, 3))
    filler_item(9, qk_group(wq_t, bq_t, qT, 1, 1, "q1t1"), ("q", 1, 1))
    filler_item(9, qk_group(wq_t, bq_t, qT, 1, 2, "q1t2"), ("q", 1, 2))
    filler_item(9, qk_group(wq_t, bq_t, qT, 1, 3, "q1t3"), ("q", 1, 3))

    def run_fillers(step, budget):
        spent = 0
        while spent < budget:
            it = None
            for x in fillers:
                if x["i"] < len(x["ops"]) and x["ready"] <= step:
                    it = x
                    break
            if it is None:
                return
            cost, fn = it["ops"][it["i"]]
            it["i"] += 1
            fn()
            spent += cost

    # ---- attention machinery ----
    CUR_STEP = [0]
    ex_store = {}
    atp_store = {}
    a2n_store = {}

    def scores_exp(pi, si):
        tb, p = PAIRS[pi]
        ensure(("k", p, si // 4))
        ensure(("q", p, tb))
        scp = scps.tile([128, 2, TB], f32, name=f"{R}sc{pi}_{si}", tag="sc")
        for h in range(2):
            nc.tensor.matmul(
                scp[:, h, :],
                kT[p][h * 64:(h + 1) * 64, si * 128:(si + 1) * 128],
                qT[p][h * 64:(h + 1) * 64, tb * TB:(tb + 1) * TB],
                start=True, stop=True)
        ex = expool.tile([128, 2, TB], bf16, name=f"{R}ex{pi}_{si}", tag="ex")
        nc.scalar.activation(ex[:], scp[:], Exp)
        ex_store[(pi, si)] = ex

    def attnv(pi, si):
        tb, p = PAIRS[pi]
        ensure(("v", si // 2))
        if si == 0:
            atp_store[pi] = atps.tile([128, 8, 128], f32,
                                      name=f"{R}atp{pi}", tag="at")
        atp = atp_store[pi]
        ex = ex_store.pop((pi, si))
        for h in range(2):
            for tcn in range(TCN):
                u = h * TCN + tcn
                # start=True zeroes the whole bank on HW: chains u=0..3
                # live in bank A (zeroed by u==0), u=4..7 in bank B
                # (zeroed by u==4); all siblings accumulate.
                nc.tensor.matmul(
                    atp[:, u, 0:65],
                    ex[:, h, tcn * 128:(tcn + 1) * 128],
                    v_aug[:, si, p * 2 + h, 0:65],
                    start=(si == 0 and u % 4 == 0), stop=(si == NS - 1),
                    skip_group_check=True)

    def normalize(pi):
        from concourse.bass import broadcast_tensor_aps
        tb, p = PAIRS[pi]
        atp = atp_store.pop(pi)
        a2n = a2pool.tile([128, TCN, 128], bf16, name=f"{R}a2n{pi}",
                          tag="a2n")
        nc.vector.reciprocal(rec8[:], atp[:, :, 64:65])
        # all 8 (h, tc) slots normalized in ONE DVE op: the reciprocal
        # column broadcasts over e via a stride-0 AP
        av = atp[:, :, 0:64].rearrange("p (h c) e -> p c h e", h=2)
        rv = rec8[:].rearrange("p (h c) o -> p c h o", h=2)
        av2, rv2 = broadcast_tensor_aps(av, rv)
        nc.vector.tensor_tensor(
            a2n[:].rearrange("p c (h e) -> p c h e", h=2), av2, rv2,
            op=Mult)
        a2n_store[pi] = a2n

    def dma_transpose(pi):
        tb, p = PAIRS[pi]
        a2n = a2n_store.pop(pi)
        for tcn in range(TCN):
            nc.sync.dma_start_transpose(
                at2[p][:, tb * TB + tcn * 128: tb * TB + (tcn + 1) * 128],
                a2n[:, tcn, :])

    # ---- main pipeline ----
    for pi in range(len(PAIRS)):
        for si in range(NS):
            gs = pi * NS + si
            CUR_STEP[0] = gs
            scores_exp(pi, si)
            if pi == len(PAIRS) - 1:
                # last pair: drain the previous pair's attnV at double
                # rate, normalize it mid-pair, then chase this pair's own
                # attnV so the tail is short.
                if si < 8:
                    attnv(pi - 1, 2 * si)
                    attnv(pi - 1, 2 * si + 1)
                    budget = 220
                elif si == 8:
                    normalize(pi - 1)
                    dma_transpose(pi - 1)
                    # out_proj for the second-to-last tb becomes filler work
                    tb_p = PAIRS[pi - 1][0]
                    for ts in range(TCN):
                        filler_item(gs + 1, out_unit(tb_p, ts))
                    budget = 430
                else:  # si 9..15: emit attnV(pi) for si 0..2*(si-9)+1
                    attnv(pi, 2 * (si - 9))
                    attnv(pi, 2 * (si - 9) + 1)
                    if si == NS - 1:
                        # exp(pi,14) is already done by now: chase one more
                        attnv(pi, 14)
                    budget = 430
            elif pi > 0:
                if si < 2:
                    # pair start: the deferred attnV waits on the previous
                    # pair's normalize (atp WAR); emit fillers first so
                    # the PE queue head is not blocked on it
                    run_fillers(gs, 430)
                    attnv(pi - 1, si)
                    budget = 0
                else:
                    attnv(pi - 1, si)
                    budget = 430
            else:
                budget = 650
            run_fillers(gs, budget)
        if 0 < pi < len(PAIRS) - 1:
            normalize(pi - 1)
            dma_transpose(pi - 1)
            if PAIRS[pi - 1][1] == 1:
                # at2 for this tb is now complete on both pairs ->
                # out-projection becomes available filler work
                tb = PAIRS[pi - 1][0]
                for ts in range(TCN):
                    filler_item(pi * NS + 1, out_unit(tb, ts))

    # ---- tail: finish last pair per t-chunk, ACT helps with drains ----
    Iden = mybir.ActivationFunctionType.Identity
    last = len(PAIRS) - 1
    tb3 = PAIRS[last][0]
    attnv(last, NS - 1)
    run_fillers(10 ** 9, 10 ** 9)  # stragglers
    from concourse.bass import broadcast_tensor_aps
    atp = atp_store.pop(last)
    a2n = a2pool.tile([128, TCN, 128], bf16, name=f"{R}a2nT", tag="a2n")
    nc.vector.reciprocal(rec8[:], atp[:, :, 64:65])
    av = atp[:, :, 0:64].rearrange("p (h c) e -> p c h e", h=2)
    rv = rec8[:].rearrange("p (h c) o -> p c h o", h=2)
    av2, rv2 = broadcast_tensor_aps(av, rv)
    nc.vector.tensor_tensor(
        a2n[:].rearrange("p c (h e) -> p c h e", h=2), av2, rv2, op=Mult)
    for tcn in range(TCN):
        nc.sync.dma_start_transpose(
            at2[1][:, tb3 * TB + tcn * 128: tb3 * TB + (tcn + 1) * 128],
            a2n[:, tcn, :])
        # out slice ts=tcn: dc banks rotate over op0/op1 and the two
        # (now idle) scores banks so units pipeline
        if tcn % 2 == 0:
            psd = [opps.tile([128, TB], f32, name=f"{R}ot{tcn}_{dc}",
                             tag=next_tag()) for dc in range(2)]
        else:
            scpair = scps.tile([128, 2, TB], f32, name=f"{R}ot{tcn}",
                               tag="sc")
            psd = [scpair[:, 0, :], scpair[:, 1, :]]
        osb = ospool.tile([128, D], bf16, name=f"{R}ot{tcn}sb", tag="os")
        for dc in range(2):
            for p in range(2):
                nc.tensor.matmul(
                    psd[dc][:],
                    at2[p][:, tb3 * TB + tcn * 128:
                           tb3 * TB + tcn * 128 + 128],
                    wo_t[:, p, dc * TB:(dc + 1) * TB],
                    start=(p == 0), stop=(p == 1))
            if dc == 0:
                nc.vector.tensor_copy(osb[:, 0:TB], psd[0][:])
            else:
                nc.scalar.activation(osb[:, TB:D], psd[1][:], Iden)
        nc.sync.dma_start(
            out_d[tb3 * TB + tcn * 128: tb3 * TB + (tcn + 1) * 128, :],
            osb[:])

    octx.close()


def _get_program(reps=1):
    global _PROGRAM
    if _PROGRAM is None:
        _PROGRAM = {}
    if reps not in _PROGRAM:
        _PROGRAM[reps] = _build_program(reps)
    return _PROGRAM[reps]


def _shard_inputs(inputs):
    """Build the 8 per-core input maps from the full-problem inputs."""
    bf16 = ml_dtypes.bfloat16
    hs = np.asarray(inputs["hidden_states"], np.float32)
    pe = np.asarray(inputs["position_embeddings"], np.float32)
    Wq = np.asarray(inputs["Wq"], np.float32).reshape(D, H * HD)
    Wk = np.asarray(inputs["Wk"], np.float32).reshape(D, H * HD)
    Wv = np.asarray(inputs["Wv"], np.float32).reshape(D, H * HD)
    Wo = np.asarray(inputs["Wo"], np.float32)
    bq = np.asarray(inputs["bq"], np.float32).reshape(H * HD)
    bk = np.asarray(inputs["bk"], np.float32).reshape(H * HD)
    bv = np.asarray(inputs["bv"], np.float32).reshape(H * HD)

    h = hs + pe
    hT = [np.ascontiguousarray(h[b].T).astype(bf16) for b in range(B)]
    xT = [np.ascontiguousarray(hs[b].T).astype(bf16) for b in range(B)]

    in_maps = []
    for c in range(8):
        b, g = divmod(c, G)
        sel = slice(g * E, (g + 1) * E)
        in_maps.append({
            "hT": hT[b],
            "xT": xT[b],
            "wq": (np.ascontiguousarray(Wq[:, sel])
                   * np.float32(SCALE)).astype(bf16),
            "wk": np.ascontiguousarray(Wk[:, sel]).astype(bf16),
            "wv": np.ascontiguousarray(Wv[:, sel]).astype(bf16),
            "wo": np.ascontiguousarray(Wo[sel, :]).astype(bf16),
            "bq": (bq[sel] * np.float32(SCALE)).reshape(2, 128, 1).copy(),
            "bk": bk[sel].reshape(2, 128, 1).copy(),
            "bvr": np.tile(bv[sel][None, :], (128, 1)),
        })
    return in_maps


def _gather_outputs(results, inputs):
    bo = np.asarray(inputs["bo"], np.float32)
    out = np.empty((B, S, D), np.float32)
    for b in range(B):
        acc = results[4 * b]["out"].astype(np.float32).copy()
        for g in range(1, G):
            acc += results[4 * b + g]["out"]
        out[b] = acc + bo[None, :]
    return out


def kernel(**inputs):
    from concourse.bass_utils import run_bass_kernel_spmd

    nc = _get_program()
    in_maps = _shard_inputs(inputs)
    res = run_bass_kernel_spmd(nc, in_maps, list(range(8)))
    return _gather_outputs(res.results, inputs)


# revision 36
# speedup vs baseline: 1.5268x; 1.0335x over previous
"""Trainium2 Bass kernel for DFine multi-head attention (v2, bf16).

Problem: B=2, S=2048, D=1024, H=16 heads, HD=64.
Sharding over 8 cores: core c handles batch b=c//4 and head-group g=c%4
(4 heads). Each core computes its heads' attention and a partial
out-projection [2048, 1024]; the host sums the 4 partials per batch and
adds the output bias.

v2 design (vs fp32r baseline):
- All matmul operands bf16 (1 cyc/row at any moving size); psum f32.
- attnV swapped: stationary = exp-tile [128s x 128t], moving = v [128s, 65]
  (64 + ones column for the softmax denominator): 65-row matmuls instead of
  512-row ones -> halves attnV PE rows.
- attnV output lands [t, head_e] in psum, so the denominator is a
  per-partition scalar: reciprocal + tensor_scalar normalize, then a
  DMA transpose (xbar) produces the [e, t] layout for the out-projection.
- out-projection DMAs straight from PSUM to DRAM (no SBUF staging).
- h = x + pos precomputed on host; inputs DMAd bf16 (half the bytes).
- Static software pipeline: per si-step emit scores -> exp -> deferred
  attnV (one pair behind, so v/atp dependencies are off the critical
  path) -> projection/out_proj filler matmuls from a deadline queue.
"""

import sys
import numpy as np
import ml_dtypes

if "/opt/trn_rl_repo" not in sys.path:
    sys.path.insert(0, "/opt/trn_rl_repo")

B, S, D, H, HD = 2, 2048, 1024, 16, 64
G = 4          # heads per core
E = G * HD     # 256 per-core head width
T = S
KC = 8         # contraction chunks of 128 over D
TB = 512       # t-block
NT = T // TB   # 4
NS = T // 128  # 16 s-chunks
TCN = TB // 128  # 4 t-chunks per t-block
SCALE = HD ** -0.5

# pair order: all p=0 pairs first so kT/qT for p=1 and the second half of
# the projection work is not demanded in the first two pairs.
PAIRS = [(0, 0), (1, 0), (2, 0), (3, 0), (0, 1), (1, 1), (2, 1), (3, 1)]

_PROGRAM = None
_DBG = False


def _build_program(reps=1):
    import concourse.bacc as bacc
    import concourse.tile as tile
    from concourse import mybir

    f32 = mybir.dt.float32
    bf16 = mybir.dt.bfloat16

    nc = bacc.Bacc("TRN2", target_bir_lowering=False, debug=False)

    hT_d = nc.declare_dram_parameter("hT", [D, T], bf16, isOutput=False)
    xT_d = nc.declare_dram_parameter("xT", [D, T], bf16, isOutput=False)
    wq_d = nc.declare_dram_parameter("wq", [D, E], bf16, isOutput=False)
    wk_d = nc.declare_dram_parameter("wk", [D, E], bf16, isOutput=False)
    wv_d = nc.declare_dram_parameter("wv", [D, E], bf16, isOutput=False)
    wo_d = nc.declare_dram_parameter("wo", [E, D], bf16, isOutput=False)
    bq_d = nc.declare_dram_parameter("bq", [2, 128, 1], f32, isOutput=False)
    bk_d = nc.declare_dram_parameter("bk", [2, 128, 1], f32, isOutput=False)
    bv_d = nc.declare_dram_parameter("bvr", [128, E], f32, isOutput=False)
    out_d = nc.declare_dram_parameter("out", [T, D], bf16, isOutput=True)

    with tile.TileContext(nc) as tc:
        for rep in range(reps):
            _build_body(nc, tc, mybir, rep,
                        (hT_d, xT_d, wq_d, wk_d, wv_d, wo_d, bq_d, bk_d,
                         bv_d, out_d))

    nc.compile()
    return nc


def _build_body(nc, tc, mybir, rep, drams):
    from contextlib import ExitStack

    f32 = mybir.dt.float32
    bf16 = mybir.dt.bfloat16
    Exp = mybir.ActivationFunctionType.Exp
    Add = mybir.AluOpType.add
    Mult = mybir.AluOpType.mult
    (hT_d, xT_d, wq_d, wk_d, wv_d, wo_d, bq_d, bk_d, bv_d, out_d) = drams
    R = f"r{rep}_"

    octx = ExitStack()
    wpool = octx.enter_context(tc.tile_pool(name=f"{R}wpool", bufs=1))
    expool = octx.enter_context(tc.tile_pool(name=f"{R}expool", bufs=18))
    a2pool = octx.enter_context(tc.tile_pool(name=f"{R}a2pool", bufs=2))
    ospool = octx.enter_context(tc.tile_pool(name=f"{R}ospool", bufs=4))
    scps = octx.enter_context(tc.tile_pool(name=f"{R}scps", bufs=2,
                                           space="PSUM"))
    atps = octx.enter_context(tc.tile_pool(name=f"{R}atps", bufs=1,
                                           space="PSUM"))
    opps = octx.enter_context(tc.tile_pool(name=f"{R}opps", bufs=1,
                                           space="PSUM"))

    # ---- persistent SBUF tiles ----
    wq_t = wpool.tile([128, KC, E], bf16, name=f"{R}wq_t")
    wk_t = wpool.tile([128, KC, E], bf16, name=f"{R}wk_t")
    wv_t = wpool.tile([128, KC, E], bf16, name=f"{R}wv_t")
    wo_t = wpool.tile([128, 2, D], bf16, name=f"{R}wo_t")
    bq_t = wpool.tile([128, 2, 1], f32, name=f"{R}bq_t")
    bk_t = wpool.tile([128, 2, 1], f32, name=f"{R}bk_t")
    bv_t = wpool.tile([128, E], f32, name=f"{R}bv_t")
    hT_t = wpool.tile([128, KC, T], bf16, name=f"{R}hT_t")
    xT_t = wpool.tile([128, KC, T], bf16, name=f"{R}xT_t")
    qT = [wpool.tile([128, T], bf16, name=f"{R}qT{p}") for p in range(2)]
    kT = [wpool.tile([128, T], bf16, name=f"{R}kT{p}") for p in range(2)]
    v_aug = wpool.tile([128, NS, G, 66], bf16, name=f"{R}v_aug")
    at2 = [wpool.tile([128, T], bf16, name=f"{R}at2_{p}") for p in range(2)]
    rec8 = wpool.tile([128, 8, 1], f32, name=f"{R}rec8")
    onecol = wpool.tile([128, NS, G, 1], bf16, name=f"{R}onecol")
    id_t = wpool.tile([128, 128], bf16, name=f"{R}id_t")

    nc.gpsimd.memset(onecol[:], 1.0)
    nc.gpsimd.tensor_copy(v_aug[:, :, :, 64:65], onecol[:])
    from concourse import masks
    masks.make_identity(nc, id_t[:])

    # ---- DMA emission (SP queue, FIFO) ----
    # wk, wq first; then hT t-block 0 chunk-by-chunk with the first k/q
    # projection matmuls chasing each chunk so scores can start ~9us in.
    # DMA order tuned for the first-scores critical path: wk, two hT
    # chunks (k-matmuls start ramping the PE), then wq, the rest of the
    # chunks, and the (tiny) biases last
    nc.sync.dma_start(
        wk_t[:], wk_d[:].rearrange("(c p) e -> p c e", p=128))
    ps_k0 = opps.tile([128, TB], f32, name=f"{R}k0s0ps", tag="op0")
    ps_q0 = opps.tile([128, TB], f32, name=f"{R}q0t0ps", tag="op1")
    kmm = [lambda k=k: nc.tensor.matmul(
        ps_k0[:], wk_t[:, k, 0:128], hT_t[:, k, 0:TB],
        start=(k == 0), stop=(k == KC - 1)) for k in range(KC)]
    qmm = [lambda k=k: nc.tensor.matmul(
        ps_q0[:], wq_t[:, k, 0:128], hT_t[:, k, 0:TB],
        start=(k == 0), stop=(k == KC - 1)) for k in range(KC)]
    for k in range(2):
        nc.sync.dma_start(hT_t[:, k, 0:TB], hT_d[k * 128:(k + 1) * 128, 0:TB])
        kmm[k]()
    nc.sync.dma_start(
        wq_t[:], wq_d[:].rearrange("(c p) e -> p c e", p=128))
    for k in range(2, KC):
        nc.sync.dma_start(hT_t[:, k, 0:TB], hT_d[k * 128:(k + 1) * 128, 0:TB])
        kmm[k]()
        qmm[k - 2]()
    qmm[KC - 2]()
    qmm[KC - 1]()
    nc.sync.dma_start(bk_t[:], bk_d[:].rearrange("c p o -> p c o"))
    nc.sync.dma_start(bq_t[:], bq_d[:].rearrange("c p o -> p c o"))
    nc.sync.dma_start(bv_t[:], bv_d[:])
    # k-drain on DVE, q-drain on the (idle at startup) ACT engine so the
    # two don't serialize ahead of the first scores
    nc.vector.tensor_scalar(kT[0][:, 0:TB], ps_k0[:], bk_t[:, 0, :], None,
                            Add)
    nc.scalar.activation(qT[0][:, 0:TB], ps_q0[:],
                         mybir.ActivationFunctionType.Identity,
                         bias=bq_t[:, 0, :])

    for qd in range(1, 4):
        nc.sync.dma_start(
            hT_t[:, :, qd * TB:(qd + 1) * TB],
            hT_d[:, qd * TB:(qd + 1) * TB].rearrange("(c p) t -> p c t",
                                                     p=128))
    nc.sync.dma_start(
        wv_t[:], wv_d[:].rearrange("(c p) e -> p c e", p=128))
    for qd in range(4):
        nc.sync.dma_start(
            xT_t[:, :, qd * TB:(qd + 1) * TB],
            xT_d[:, qd * TB:(qd + 1) * TB].rearrange("(c p) t -> p c t",
                                                     p=128))
    nc.sync.dma_start(
        wo_t[:], wo_d[:].rearrange("(c p) d -> p c d", p=128))

    # ---- filler queue: deadline-ordered projection / out_proj work ----
    tag_i = [0]

    def next_tag():
        t = f"op{tag_i[0] % 2}"
        tag_i[0] += 1
        return t

    def qk_group(w_t, b_t, dstT, p, blk, nm):
        box = {}
        tag = [None]

        def mk_mm(k):
            def f():
                if k == 0:
                    tag[0] = next_tag()
                    box["ps"] = opps.tile([128, TB], f32,
                                          name=f"{R}{nm}ps", tag=tag[0])
                nc.tensor.matmul(box["ps"][:],
                                 w_t[:, k, p * 128:(p + 1) * 128],
                                 hT_t[:, k, blk * TB:(blk + 1) * TB],
                                 start=(k == 0), stop=(k == KC - 1))
            return f

        ops = [(213, mk_mm(k)) for k in range(KC)]

        def drain():
            nc.vector.tensor_scalar(dstT[p][:, blk * TB:(blk + 1) * TB],
                                    box["ps"][:], b_t[:, p, :], None, Add)
        ops.append((0, drain))
        return ops

    def v_group(j):
        # si pair (2j, 2j+1): two 8-matmul chains into one psum bank
        box = {}
        tag = [None]

        def mk_mm(k, jj):
            def f():
                if k == 0 and jj == 0:
                    tag[0] = next_tag()
                    box["ps"] = opps.tile([128, 2, E], f32,
                                          name=f"{R}v{j}ps", tag=tag[0])
                si = 2 * j + jj
                # HW: start=True zeroes the whole psum bank, so only the
                # first chain in the bank starts; the sibling accumulates.
                nc.tensor.matmul(box["ps"][:, jj, :],
                                 xT_t[:, k, si * 128:(si + 1) * 128],
                                 wv_t[:, k, :],
                                 start=(k == 0 and jj == 0),
                                 stop=(k == KC - 1),
                                 skip_group_check=True)
            return f

        ops = []
        for k in range(KC):
            for jj in range(2):
                ops.append((107, mk_mm(k, jj)))

        def mk_drain(jj):
            def f():
                si = 2 * j + jj
                nc.vector.tensor_tensor(
                    v_aug[:, si, :, 0:64],
                    box["ps"][:, jj, :].rearrange("p (g e) -> p g e", g=G),
                    bv_t[:].rearrange("p (g e) -> p g e", g=G),
                    op=Add)
            return f
        ops.append((0, mk_drain(0)))
        ops.append((0, mk_drain(1)))
        return ops

    def out_unit(tb, ts):
        # one 128-row output slice: dc0 fills one psum bank, dc1 the
        # other; both drain into one bf16 staging row -> single DMA.
        box = {}

        def mk_mm(dc, p):
            def f():
                if p == 0:
                    box[dc] = opps.tile([128, TB], f32,
                                        name=f"{R}o{tb}_{ts}_{dc}ps",
                                        tag=next_tag())
                nc.tensor.matmul(box[dc][:],
                                 at2[p][:, tb * TB + ts * 128:
                                        tb * TB + ts * 128 + 128],
                                 wo_t[:, p, dc * TB:(dc + 1) * TB],
                                 start=(p == 0), stop=(p == 1))
            return f

        def mk_drain(dc):
            def f():
                if dc == 0:
                    box["osb"] = ospool.tile([128, D], bf16,
                                             name=f"{R}o{tb}_{ts}sb",
                                             tag="os")
                nc.vector.tensor_copy(
                    box["osb"][:, dc * TB:(dc + 1) * TB], box[dc][:])
            return f

        def dma():
            nc.sync.dma_start(
                out_d[tb * TB + ts * 128: tb * TB + (ts + 1) * 128, :],
                box["osb"][:])
        return [(213, mk_mm(0, 0)), (213, mk_mm(0, 1)), (0, mk_drain(0)),
                (213, mk_mm(1, 0)), (213, mk_mm(1, 1)), (0, mk_drain(1)),
                (0, dma)]

    fillers = []
    filler_by_key = {}

    def filler_item(ready, ops, key=None):
        it = {"ready": ready, "ops": ops, "i": 0, "key": key}
        fillers.append(it)
        if key is not None:
            filler_by_key[key] = it

    def ensure(key, _dbg=[0]):
        """Force-emit every remaining op of the filler item `key` so a
        consumer emitted next observes its writes (tile deps only order
        instructions that are already emitted)."""
        it = filler_by_key.get(key)
        if it is None:
            return
        n = len(it["ops"]) - it["i"]
        if n > 0 and _DBG:
            print(f"ENSURE {key} forces {n} ops at step {CUR_STEP[0]}")
        while it["i"] < len(it["ops"]):
            _, fn = it["ops"][it["i"]]
            it["i"] += 1
            fn()

    # deadline-ordered: (the scan picks the first *ready* item)
    filler_item(1, qk_group(wk_t, bk_t, kT, 0, 1, "k0s1"), ("k", 0, 1))
    filler_item(4, qk_group(wk_t, bk_t, kT, 0, 2, "k0s2"), ("k", 0, 2))
    filler_item(7, qk_group(wk_t, bk_t, kT, 0, 3, "k0s3"), ("k", 0, 3))
    filler_item(2, qk_group(wq_t, bq_t, qT, 0, 1, "q0t1"), ("q", 0, 1))
    filler_item(4, qk_group(wq_t, bq_t, qT, 0, 2, "q0t2"), ("q", 0, 2))
    filler_item(11, v_group(0), ("v", 0))
    filler_item(11, v_group(1), ("v", 1))
    filler_item(14, v_group(2), ("v", 2))
    filler_item(14, v_group(3), ("v", 3))
    filler_item(17, v_group(4), ("v", 4))
    filler_item(17, v_group(5), ("v", 5))
    filler_item(20, v_group(6), ("v", 6))
    filler_item(20, v_group(7), ("v", 7))
    filler_item(7, qk_group(wq_t, bq_t, qT, 0, 3, "q0t3"), ("q", 0, 3))
    filler_item(28, qk_group(wk_t, bk_t, kT, 1, 0, "k1s0"), ("k", 1, 0))
    filler_item(31, qk_group(wq_t, bq_t, qT, 1, 0, "q1t0"), ("q", 1, 0))
    filler_item(40, qk_group(wk_t, bk_t, kT, 1, 1, "k1s1"), ("k", 1, 1))
    filler_item(48, qk_group(wk_t, bk_t, kT, 1, 2, "k1s2"), ("k", 1, 2))
    filler_item(56, qk_group(wk_t, bk_t, kT, 1, 3, "k1s3"), ("k", 1, 3))
    filler_item(60, qk_group(wq_t, bq_t, qT, 1, 1, "q1t1"), ("q", 1, 1))
    filler_item(64, qk_group(wq_t, bq_t, qT, 1, 2, "q1t2"), ("q", 1, 2))
    filler_item(68, qk_group(wq_t, bq_t, qT, 1, 3, "q1t3"), ("q", 1, 3))

    def run_fillers(step, budget):
        spent = 0
        while spent < budget:
            it = None
            for x in fillers:
                if x["i"] < len(x["ops"]) and x["ready"] <= step:
                    it = x
                    break
            if it is None:
                return
            cost, fn = it["ops"][it["i"]]
            it["i"] += 1
            fn()
            spent += cost

    # ---- attention machinery ----
    CUR_STEP = [0]
    ex_store = {}
    atp_store = {}
    a2n_store = {}

    def scores_exp(pi, si):
        tb, p = PAIRS[pi]
        ensure(("k", p, si // 4))
        ensure(("q", p, tb))
        scp = scps.tile([128, 2, TB], f32, name=f"{R}sc{pi}_{si}", tag="sc")
        for h in range(2):
            nc.tensor.matmul(
                scp[:, h, :],
                kT[p][h * 64:(h + 1) * 64, si * 128:(si + 1) * 128],
                qT[p][h * 64:(h + 1) * 64, tb * TB:(tb + 1) * TB],
                start=True, stop=True)
        ex = expool.tile([128, 2, TB], bf16, name=f"{R}ex{pi}_{si}", tag="ex")
        if pi == len(PAIRS) - 1 and si == NS - 1:
            # the very last exp gates the tail: split by head so the h0
            # attnV chains (and everything after) start half an exp early
            nc.scalar.activation(ex[:, 0, :], scp[:, 0, :], Exp)
            nc.scalar.activation(ex[:, 1, :], scp[:, 1, :], Exp)
        else:
            nc.scalar.activation(ex[:], scp[:], Exp)
        ex_store[(pi, si)] = ex

    def attnv(pi, si):
        tb, p = PAIRS[pi]
        ensure(("v", si // 2))
        if si == 0:
            atp_store[pi] = atps.tile([128, 8, 128], f32,
                                      name=f"{R}atp{pi}", tag="at")
        atp = atp_store[pi]
        ex = ex_store.pop((pi, si))
        for h in range(2):
            for tcn in range(TCN):
                u = h * TCN + tcn
                # start=True zeroes the whole bank on HW: chains u=0..3
                # live in bank A (zeroed by u==0), u=4..7 in bank B
                # (zeroed by u==4); all siblings accumulate.
                nc.tensor.matmul(
                    atp[:, u, 0:65],
                    ex[:, h, tcn * 128:(tcn + 1) * 128],
                    v_aug[:, si, p * 2 + h, 0:65],
                    start=(si == 0 and u % 4 == 0), stop=(si == NS - 1),
                    skip_group_check=True)

    def normalize(pi):
        from concourse.bass import broadcast_tensor_aps
        tb, p = PAIRS[pi]
        atp = atp_store.pop(pi)
        a2n = a2pool.tile([128, TCN, 128], bf16, name=f"{R}a2n{pi}",
                          tag="a2n")
        nc.vector.reciprocal(rec8[:], atp[:, :, 64:65])
        # all 8 (h, tc) slots normalized in ONE DVE op: the reciprocal
        # column broadcasts over e via a stride-0 AP
        av = atp[:, :, 0:64].rearrange("p (h c) e -> p c h e", h=2)
        rv = rec8[:].rearrange("p (h c) o -> p c h o", h=2)
        av2, rv2 = broadcast_tensor_aps(av, rv)
        nc.vector.tensor_tensor(
            a2n[:].rearrange("p c (h e) -> p c h e", h=2), av2, rv2,
            op=Mult)
        a2n_store[pi] = a2n

    def dma_transpose(pi):
        tb, p = PAIRS[pi]
        a2n = a2n_store.pop(pi)
        for tcn in range(TCN):
            nc.sync.dma_start_transpose(
                at2[p][:, tb * TB + tcn * 128: tb * TB + (tcn + 1) * 128],
                a2n[:, tcn, :])

    # ---- main pipeline ----
    for pi in range(len(PAIRS)):
        for si in range(NS):
            gs = pi * NS + si
            CUR_STEP[0] = gs
            scores_exp(pi, si)
            if pi == len(PAIRS) - 1:
                # last pair: drain the previous pair's attnV at double
                # rate, normalize it mid-pair, then chase this pair's own
                # attnV so the tail is short.
                if si < 8:
                    attnv(pi - 1, 2 * si)
                    attnv(pi - 1, 2 * si + 1)
                    budget = 220
                elif si == 8:
                    normalize(pi - 1)
                    dma_transpose(pi - 1)
                    # half of the second-to-last tb's out_proj runs here;
                    # the other half fills the tail's dead PE time
                    tb_p = PAIRS[pi - 1][0]
                    for ts in range(2):
                        filler_item(gs + 1, out_unit(tb_p, ts))
                    budget = 300
                else:  # si 9..15: emit attnV(pi) for si 0..2*(si-9)+1
                    attnv(pi, 2 * (si - 9))
                    attnv(pi, 2 * (si - 9) + 1)
                    if si == NS - 1:
                        # exp(pi,14) is already done by now: chase one more
                        attnv(pi, 14)
                    budget = 300
            elif pi > 0:
                if si < 2:
                    # pair start: the deferred attnV waits on the previous
                    # pair's normalize (atp WAR); emit fillers first so
                    # the PE queue head is not blocked on it
                    run_fillers(gs, 430)
                    attnv(pi - 1, si)
                    budget = 0
                else:
                    attnv(pi - 1, si)
                    budget = 430
            else:
                budget = 650
            run_fillers(gs, budget)
        if 0 < pi < len(PAIRS) - 1:
            normalize(pi - 1)
            dma_transpose(pi - 1)
            if PAIRS[pi - 1][1] == 1:
                # at2 for this tb is now complete on both pairs ->
                # out-projection becomes available filler work
                tb = PAIRS[pi - 1][0]
                for ts in range(TCN):
                    filler_item(pi * NS + 1, out_unit(tb, ts))

    # ---- tail: finish last pair per t-chunk, ACT helps with drains ----
    Iden = mybir.ActivationFunctionType.Identity
    last = len(PAIRS) - 1
    tb3 = PAIRS[last][0]
    attnv(last, NS - 1)
    run_fillers(10 ** 9, 10 ** 9)  # stragglers
    from concourse.bass import broadcast_tensor_aps
    atp = atp_store.pop(last)
    a2n = a2pool.tile([128, TCN, 128], bf16, name=f"{R}a2nT", tag="a2n")
    nc.vector.reciprocal(rec8[:], atp[:, :, 64:65])
    av = atp[:, :, 0:64].rearrange("p (h c) e -> p c h e", h=2)
    rv = rec8[:].rearrange("p (h c) o -> p c h o", h=2)
    av2, rv2 = broadcast_tensor_aps(av, rv)
    nc.vector.tensor_tensor(
        a2n[:].rearrange("p c (h e) -> p c h e", h=2), av2, rv2, op=Mult)
    # PE-transpose into psum (53ns each) + copy instead of DMA-transpose:
    # saves the ~2.3us DGE/sem latency on the tail critical path.
    # Phase-ordered emission (all transposes -> all copies -> fills with
    # drains/DMAs chasing) so the in-order PE queue never interleaves a
    # stalled op ahead of ready fills.
    tp = atps.tile([128, TCN, 128], bf16, name=f"{R}tpT", tag="at")
    for tcn in range(TCN):
        nc.tensor.matmul(tp[:, tcn, :], a2n[:, tcn, :], id_t[:],
                         is_transpose=True, start=(tcn == 0), stop=True,
                         skip_group_check=True)
    for tcn in range(TCN):
        at2s = at2[1][:, tb3 * TB + tcn * 128: tb3 * TB + (tcn + 1) * 128]
        if tcn % 2 == 0:
            nc.vector.tensor_copy(at2s, tp[:, tcn, :])
        else:
            nc.scalar.activation(at2s, tp[:, tcn, :], Iden)
    psds, osbs = [], []
    for tcn in range(TCN):
        if tcn % 2 == 0:
            psd = [opps.tile([128, TB], f32, name=f"{R}ot{tcn}_{dc}",
                             tag=next_tag()) for dc in range(2)]
        else:
            scpair = scps.tile([128, 2, TB], f32, name=f"{R}ot{tcn}",
                               tag="sc")
            psd = [scpair[:, 0, :], scpair[:, 1, :]]
        psds.append(psd)
        osbs.append(ospool.tile([128, D], bf16, name=f"{R}ot{tcn}sb",
                                tag="os"))
    for tcn in range(TCN):
        for dc in range(2):
            for p in range(2):
                nc.tensor.matmul(
                    psds[tcn][dc][:],
                    at2[p][:, tb3 * TB + tcn * 128:
                           tb3 * TB + tcn * 128 + 128],
                    wo_t[:, p, dc * TB:(dc + 1) * TB],
                    start=(p == 0), stop=(p == 1))
            if dc == 0:
                nc.vector.tensor_copy(osbs[tcn][:, 0:TB], psds[tcn][0][:])
            else:
                nc.scalar.activation(osbs[tcn][:, TB:D], psds[tcn][1][:],
                                     Iden)
        nc.sync.dma_start(
            out_d[tb3 * TB + tcn * 128: tb3 * TB + (tcn + 1) * 128, :],
            osbs[tcn][:])

    octx.close()


def _get_program(reps=1):
    global _PROGRAM
    if _PROGRAM is None:
        _PROGRAM = {}
    if reps not in _PROGRAM:
        _PROGRAM[reps] = _build_program(reps)
    return _PROGRAM[reps]


def _shard_inputs(inputs):
    """Build the 8 per-core input maps from the full-problem inputs."""
    bf16 = ml_dtypes.bfloat16
    hs = np.asarray(inputs["hidden_states"], np.float32)
    pe = np.asarray(inputs["position_embeddings"], np.float32)
    Wq = np.asarray(inputs["Wq"], np.float32).reshape(D, H * HD)
    Wk = np.asarray(inputs["Wk"], np.float32).reshape(D, H * HD)
    Wv = np.asarray(inputs["Wv"], np.float32).reshape(D, H * HD)
    Wo = np.asarray(inputs["Wo"], np.float32)
    bq = np.asarray(inputs["bq"], np.float32).reshape(H * HD)
    bk = np.asarray(inputs["bk"], np.float32).reshape(H * HD)
    bv = np.asarray(inputs["bv"], np.float32).reshape(H * HD)

    h = hs + pe
    hT = [np.ascontiguousarray(h[b].T).astype(bf16) for b in range(B)]
    xT = [np.ascontiguousarray(hs[b].T).astype(bf16) for b in range(B)]

    in_maps = []
    for c in range(8):
        b, g = divmod(c, G)
        sel = slice(g * E, (g + 1) * E)
        in_maps.append({
            "hT": hT[b],
            "xT": xT[b],
            "wq": (np.ascontiguousarray(Wq[:, sel])
                   * np.float32(SCALE)).astype(bf16),
            "wk": np.ascontiguousarray(Wk[:, sel]).astype(bf16),
            "wv": np.ascontiguousarray(Wv[:, sel]).astype(bf16),
            "wo": np.ascontiguousarray(Wo[sel, :]).astype(bf16),
            "bq": (bq[sel] * np.float32(SCALE)).reshape(2, 128, 1).copy(),
            "bk": bk[sel].reshape(2, 128, 1).copy(),
            "bvr": np.tile(bv[sel][None, :], (128, 1)),
        })
    return in_maps


def _gather_outputs(results, inputs):
    bo = np.asarray(inputs["bo"], np.float32)
    out = np.empty((B, S, D), np.float32)
    for b in range(B):
        acc = results[4 * b]["out"].astype(np.float32).copy()
        for g in range(1, G):
            acc += results[4 * b + g]["out"]
        out[b] = acc + bo[None, :]
    return out


def kernel(**inputs):
    from concourse.bass_utils import run_bass_kernel_spmd

    nc = _get_program()
    in_maps = _shard_inputs(inputs)
    res = run_bass_kernel_spmd(nc, in_maps, list(range(8)))
    return _gather_outputs(res.results, inputs)


# revision 40
# speedup vs baseline: 1.5357x; 1.0058x over previous
"""Trainium2 Bass kernel for DFine multi-head attention (v2, bf16).

Problem: B=2, S=2048, D=1024, H=16 heads, HD=64.
Sharding over 8 cores: core c handles batch b=c//4 and head-group g=c%4
(4 heads). Each core computes its heads' attention and a partial
out-projection [2048, 1024]; the host sums the 4 partials per batch and
adds the output bias.

v2 design (vs fp32r baseline):
- All matmul operands bf16 (1 cyc/row at any moving size); psum f32.
- attnV swapped: stationary = exp-tile [128s x 128t], moving = v [128s, 65]
  (64 + ones column for the softmax denominator): 65-row matmuls instead of
  512-row ones -> halves attnV PE rows.
- attnV output lands [t, head_e] in psum, so the denominator is a
  per-partition scalar: reciprocal + tensor_scalar normalize, then a
  DMA transpose (xbar) produces the [e, t] layout for the out-projection.
- out-projection DMAs straight from PSUM to DRAM (no SBUF staging).
- h = x + pos precomputed on host; inputs DMAd bf16 (half the bytes).
- Static software pipeline: per si-step emit scores -> exp -> deferred
  attnV (one pair behind, so v/atp dependencies are off the critical
  path) -> projection/out_proj filler matmuls from a deadline queue.
"""

import sys
import numpy as np
import ml_dtypes

if "/opt/trn_rl_repo" not in sys.path:
    sys.path.insert(0, "/opt/trn_rl_repo")

B, S, D, H, HD = 2, 2048, 1024, 16, 64
G = 4          # heads per core
E = G * HD     # 256 per-core head width
T = S
KC = 8         # contraction chunks of 128 over D
TB = 512       # t-block
NT = T // TB   # 4
NS = T // 128  # 16 s-chunks
TCN = TB // 128  # 4 t-chunks per t-block
SCALE = HD ** -0.5

# pair order: all p=0 pairs first so kT/qT for p=1 and the second half of
# the projection work is not demanded in the first two pairs.
PAIRS = [(0, 0), (1, 0), (2, 0), (3, 0), (0, 1), (1, 1), (2, 1), (3, 1)]

_PROGRAM = None
_DBG = False


def _build_program(reps=1):
    import concourse.bacc as bacc
    import concourse.tile as tile
    from concourse import mybir

    f32 = mybir.dt.float32
    bf16 = mybir.dt.bfloat16

    nc = bacc.Bacc("TRN2", target_bir_lowering=False, debug=False)

    hT_d = nc.declare_dram_parameter("hT", [D, T], bf16, isOutput=False)
    xT_d = nc.declare_dram_parameter("xT", [D, T], bf16, isOutput=False)
    wq_d = nc.declare_dram_parameter("wq", [D, E], bf16, isOutput=False)
    wk_d = nc.declare_dram_parameter("wk", [D, E], bf16, isOutput=False)
    wv_d = nc.declare_dram_parameter("wv", [D, E], bf16, isOutput=False)
    wo_d = nc.declare_dram_parameter("wo", [E, D], bf16, isOutput=False)
    bq_d = nc.declare_dram_parameter("bq", [2, 128, 1], f32, isOutput=False)
    bk_d = nc.declare_dram_parameter("bk", [2, 128, 1], f32, isOutput=False)
    bv_d = nc.declare_dram_parameter("bvr", [128, E], f32, isOutput=False)
    out_d = nc.declare_dram_parameter("out", [T, D], bf16, isOutput=True)

    with tile.TileContext(nc) as tc:
        for rep in range(reps):
            _build_body(nc, tc, mybir, rep,
                        (hT_d, xT_d, wq_d, wk_d, wv_d, wo_d, bq_d, bk_d,
                         bv_d, out_d))

    nc.compile()
    return nc


def _build_body(nc, tc, mybir, rep, drams):
    from contextlib import ExitStack

    f32 = mybir.dt.float32
    bf16 = mybir.dt.bfloat16
    Exp = mybir.ActivationFunctionType.Exp
    Add = mybir.AluOpType.add
    Mult = mybir.AluOpType.mult
    (hT_d, xT_d, wq_d, wk_d, wv_d, wo_d, bq_d, bk_d, bv_d, out_d) = drams
    R = f"r{rep}_"

    octx = ExitStack()
    wpool = octx.enter_context(tc.tile_pool(name=f"{R}wpool", bufs=1))
    expool = octx.enter_context(tc.tile_pool(name=f"{R}expool", bufs=18))
    a2pool = octx.enter_context(tc.tile_pool(name=f"{R}a2pool", bufs=2))
    ospool = octx.enter_context(tc.tile_pool(name=f"{R}ospool", bufs=4))
    scps = octx.enter_context(tc.tile_pool(name=f"{R}scps", bufs=2,
                                           space="PSUM"))
    atps = octx.enter_context(tc.tile_pool(name=f"{R}atps", bufs=1,
                                           space="PSUM"))
    opps = octx.enter_context(tc.tile_pool(name=f"{R}opps", bufs=1,
                                           space="PSUM"))

    # ---- persistent SBUF tiles ----
    wq_t = wpool.tile([128, KC, E], bf16, name=f"{R}wq_t")
    wk_t = wpool.tile([128, KC, E], bf16, name=f"{R}wk_t")
    wv_t = wpool.tile([128, KC, E], bf16, name=f"{R}wv_t")
    wo_t = wpool.tile([128, 2, D], bf16, name=f"{R}wo_t")
    bq_t = wpool.tile([128, 2, 1], f32, name=f"{R}bq_t")
    bk_t = wpool.tile([128, 2, 1], f32, name=f"{R}bk_t")
    bv_t = wpool.tile([128, E], f32, name=f"{R}bv_t")
    hT_t = wpool.tile([128, KC, T], bf16, name=f"{R}hT_t")
    xT_t = wpool.tile([128, KC, T], bf16, name=f"{R}xT_t")
    qT = [wpool.tile([128, T], bf16, name=f"{R}qT{p}") for p in range(2)]
    kT = [wpool.tile([128, T], bf16, name=f"{R}kT{p}") for p in range(2)]
    v_aug = wpool.tile([128, NS, G, 66], bf16, name=f"{R}v_aug")
    at2 = [wpool.tile([128, T], bf16, name=f"{R}at2_{p}") for p in range(2)]
    rec8 = wpool.tile([128, 8, 1], f32, name=f"{R}rec8")
    onecol = wpool.tile([128, NS, G, 1], bf16, name=f"{R}onecol")
    id_t = wpool.tile([128, 128], bf16, name=f"{R}id_t")

    nc.gpsimd.memset(onecol[:], 1.0)
    nc.gpsimd.tensor_copy(v_aug[:, :, :, 64:65], onecol[:])
    from concourse import masks
    masks.make_identity(nc, id_t[:])
    # dummy exp at t=0 so the 1.3us activation-table load happens under
    # the input DMAs instead of right before the first real exp
    warm = wpool.tile([1, 1], f32, name=f"{R}warm")
    nc.scalar.activation(warm[:], warm[:], Exp)

    # ---- DMA emission (SP queue, FIFO) ----
    # wk, wq first; then hT t-block 0 chunk-by-chunk with the first k/q
    # projection matmuls chasing each chunk so scores can start ~9us in.
    # DMA order tuned for the first-scores critical path: wk, two hT
    # chunks (k-matmuls start ramping the PE), then wq, the rest of the
    # chunks, and the (tiny) biases last
    nc.sync.dma_start(
        wk_t[:], wk_d[:].rearrange("(c p) e -> p c e", p=128))
    ps_k0 = opps.tile([128, TB], f32, name=f"{R}k0s0ps", tag="op0")
    ps_q0 = opps.tile([128, TB], f32, name=f"{R}q0t0ps", tag="op1")
    kmm = [lambda k=k: nc.tensor.matmul(
        ps_k0[:], wk_t[:, k, 0:128], hT_t[:, k, 0:TB],
        start=(k == 0), stop=(k == KC - 1)) for k in range(KC)]
    qmm = [lambda k=k: nc.tensor.matmul(
        ps_q0[:], wq_t[:, k, 0:128], hT_t[:, k, 0:TB],
        start=(k == 0), stop=(k == KC - 1)) for k in range(KC)]
    for k in range(2):
        nc.sync.dma_start(hT_t[:, k, 0:TB], hT_d[k * 128:(k + 1) * 128, 0:TB])
        kmm[k]()
    nc.sync.dma_start(
        wq_t[:], wq_d[:].rearrange("(c p) e -> p c e", p=128))
    for k in range(2, KC):
        nc.sync.dma_start(hT_t[:, k, 0:TB], hT_d[k * 128:(k + 1) * 128, 0:TB])
        kmm[k]()
        qmm[k - 2]()
    qmm[KC - 2]()
    qmm[KC - 1]()
    nc.sync.dma_start(bk_t[:], bk_d[:].rearrange("c p o -> p c o"))
    nc.sync.dma_start(bq_t[:], bq_d[:].rearrange("c p o -> p c o"))
    nc.sync.dma_start(bv_t[:], bv_d[:])
    # k-drain on DVE, q-drain on the (idle at startup) ACT engine so the
    # two don't serialize ahead of the first scores
    nc.vector.tensor_scalar(kT[0][:, 0:TB], ps_k0[:], bk_t[:, 0, :], None,
                            Add)
    nc.scalar.activation(qT[0][:, 0:TB], ps_q0[:],
                         mybir.ActivationFunctionType.Identity,
                         bias=bq_t[:, 0, :])

    def _hq(qd):
        nc.sync.dma_start(
            hT_t[:, :, qd * TB:(qd + 1) * TB],
            hT_d[:, qd * TB:(qd + 1) * TB].rearrange("(c p) t -> p c t",
                                                     p=128))

    def _xq(qd):
        nc.sync.dma_start(
            xT_t[:, :, qd * TB:(qd + 1) * TB],
            xT_d[:, qd * TB:(qd + 1) * TB].rearrange("(c p) t -> p c t",
                                                     p=128))

    _hq(1)
    nc.sync.dma_start(
        wv_t[:], wv_d[:].rearrange("(c p) e -> p c e", p=128))
    _xq(0)
    _hq(2)
    _xq(1)
    _hq(3)
    _xq(2)
    _xq(3)
    nc.sync.dma_start(
        wo_t[:], wo_d[:].rearrange("(c p) d -> p c d", p=128))

    # ---- filler queue: deadline-ordered projection / out_proj work ----
    tag_i = [0]

    def next_tag():
        t = f"op{tag_i[0] % 2}"
        tag_i[0] += 1
        return t

    def qk_group(w_t, b_t, dstT, p, blk, nm):
        box = {}
        tag = [None]

        def mk_mm(k):
            def f():
                if k == 0:
                    tag[0] = next_tag()
                    box["ps"] = opps.tile([128, TB], f32,
                                          name=f"{R}{nm}ps", tag=tag[0])
                nc.tensor.matmul(box["ps"][:],
                                 w_t[:, k, p * 128:(p + 1) * 128],
                                 hT_t[:, k, blk * TB:(blk + 1) * TB],
                                 start=(k == 0), stop=(k == KC - 1))
            return f

        ops = [(213, mk_mm(k)) for k in range(KC)]

        def drain():
            nc.vector.tensor_scalar(dstT[p][:, blk * TB:(blk + 1) * TB],
                                    box["ps"][:], b_t[:, p, :], None, Add)
        ops.append((0, drain))
        return ops

    def v_group(j):
        # si pair (2j, 2j+1): two 8-matmul chains into one psum bank
        box = {}
        tag = [None]

        def mk_mm(k, jj):
            def f():
                if k == 0 and jj == 0:
                    tag[0] = next_tag()
                    box["ps"] = opps.tile([128, 2, E], f32,
                                          name=f"{R}v{j}ps", tag=tag[0])
                si = 2 * j + jj
                # HW: start=True zeroes the whole psum bank, so only the
                # first chain in the bank starts; the sibling accumulates.
                nc.tensor.matmul(box["ps"][:, jj, :],
                                 xT_t[:, k, si * 128:(si + 1) * 128],
                                 wv_t[:, k, :],
                                 start=(k == 0 and jj == 0),
                                 stop=(k == KC - 1),
                                 skip_group_check=True)
            return f

        ops = []
        for k in range(KC):
            for jj in range(2):
                ops.append((107, mk_mm(k, jj)))

        def mk_drain(jj):
            def f():
                si = 2 * j + jj
                nc.vector.tensor_tensor(
                    v_aug[:, si, :, 0:64],
                    box["ps"][:, jj, :].rearrange("p (g e) -> p g e", g=G),
                    bv_t[:].rearrange("p (g e) -> p g e", g=G),
                    op=Add)
            return f
        ops.append((0, mk_drain(0)))
        ops.append((0, mk_drain(1)))
        return ops

    def out_unit(tb, ts):
        # one 128-row output slice: dc0 fills one psum bank, dc1 the
        # other; both drain into one bf16 staging row -> single DMA.
        box = {}

        def mk_mm(dc, p):
            def f():
                if p == 0:
                    box[dc] = opps.tile([128, TB], f32,
                                        name=f"{R}o{tb}_{ts}_{dc}ps",
                                        tag=next_tag())
                nc.tensor.matmul(box[dc][:],
                                 at2[p][:, tb * TB + ts * 128:
                                        tb * TB + ts * 128 + 128],
                                 wo_t[:, p, dc * TB:(dc + 1) * TB],
                                 start=(p == 0), stop=(p == 1))
            return f

        def mk_drain(dc):
            def f():
                if dc == 0:
                    box["osb"] = ospool.tile([128, D], bf16,
                                             name=f"{R}o{tb}_{ts}sb",
                                             tag="os")
                nc.vector.tensor_copy(
                    box["osb"][:, dc * TB:(dc + 1) * TB], box[dc][:])
            return f

        def dma():
            nc.sync.dma_start(
                out_d[tb * TB + ts * 128: tb * TB + (ts + 1) * 128, :],
                box["osb"][:])
        return [(213, mk_mm(0, 0)), (213, mk_mm(0, 1)), (0, mk_drain(0)),
                (213, mk_mm(1, 0)), (213, mk_mm(1, 1)), (0, mk_drain(1)),
                (0, dma)]

    fillers = []
    filler_by_key = {}

    def filler_item(ready, ops, key=None):
        it = {"ready": ready, "ops": ops, "i": 0, "key": key}
        fillers.append(it)
        if key is not None:
            filler_by_key[key] = it

    def ensure(key, _dbg=[0]):
        """Force-emit every remaining op of the filler item `key` so a
        consumer emitted next observes its writes (tile deps only order
        instructions that are already emitted)."""
        it = filler_by_key.get(key)
        if it is None:
            return
        n = len(it["ops"]) - it["i"]
        if n > 0 and _DBG:
            print(f"ENSURE {key} forces {n} ops at step {CUR_STEP[0]}")
        while it["i"] < len(it["ops"]):
            _, fn = it["ops"][it["i"]]
            it["i"] += 1
            fn()

    # deadline-ordered: (the scan picks the first *ready* item)
    filler_item(1, qk_group(wk_t, bk_t, kT, 0, 1, "k0s1"), ("k", 0, 1))
    filler_item(4, qk_group(wk_t, bk_t, kT, 0, 2, "k0s2"), ("k", 0, 2))
    filler_item(7, qk_group(wk_t, bk_t, kT, 0, 3, "k0s3"), ("k", 0, 3))
    filler_item(2, qk_group(wq_t, bq_t, qT, 0, 1, "q0t1"), ("q", 0, 1))
    filler_item(4, qk_group(wq_t, bq_t, qT, 0, 2, "q0t2"), ("q", 0, 2))
    filler_item(4, v_group(0), ("v", 0))
    filler_item(4, v_group(1), ("v", 1))
    filler_item(9, v_group(2), ("v", 2))
    filler_item(9, v_group(3), ("v", 3))
    filler_item(15, v_group(4), ("v", 4))
    filler_item(15, v_group(5), ("v", 5))
    filler_item(18, v_group(6), ("v", 6))
    filler_item(18, v_group(7), ("v", 7))
    filler_item(7, qk_group(wq_t, bq_t, qT, 0, 3, "q0t3"), ("q", 0, 3))
    filler_item(28, qk_group(wk_t, bk_t, kT, 1, 0, "k1s0"), ("k", 1, 0))
    filler_item(31, qk_group(wq_t, bq_t, qT, 1, 0, "q1t0"), ("q", 1, 0))
    filler_item(40, qk_group(wk_t, bk_t, kT, 1, 1, "k1s1"), ("k", 1, 1))
    filler_item(48, qk_group(wk_t, bk_t, kT, 1, 2, "k1s2"), ("k", 1, 2))
    filler_item(56, qk_group(wk_t, bk_t, kT, 1, 3, "k1s3"), ("k", 1, 3))
    filler_item(60, qk_group(wq_t, bq_t, qT, 1, 1, "q1t1"), ("q", 1, 1))
    filler_item(64, qk_group(wq_t, bq_t, qT, 1, 2, "q1t2"), ("q", 1, 2))
    filler_item(68, qk_group(wq_t, bq_t, qT, 1, 3, "q1t3"), ("q", 1, 3))

    def run_fillers(step, budget):
        spent = 0
        while spent < budget:
            it = None
            for x in fillers:
                if x["i"] < len(x["ops"]) and x["ready"] <= step:
                    it = x
                    break
            if it is None:
                return
            cost, fn = it["ops"][it["i"]]
            it["i"] += 1
            fn()
            spent += cost

    # ---- attention machinery ----
    CUR_STEP = [0]
    ex_store = {}
    atp_store = {}
    a2n_store = {}

    def scores_exp(pi, si):
        tb, p = PAIRS[pi]
        ensure(("k", p, si // 4))
        ensure(("q", p, tb))
        scp = scps.tile([128, 2, TB], f32, name=f"{R}sc{pi}_{si}", tag="sc")
        for h in range(2):
            nc.tensor.matmul(
                scp[:, h, :],
                kT[p][h * 64:(h + 1) * 64, si * 128:(si + 1) * 128],
                qT[p][h * 64:(h + 1) * 64, tb * TB:(tb + 1) * TB],
                start=True, stop=True)
        ex = expool.tile([128, 2, TB], bf16, name=f"{R}ex{pi}_{si}", tag="ex")
        if pi == len(PAIRS) - 1 and si == NS - 1:
            # the very last exp gates the tail: split by head so the h0
            # attnV chains (and everything after) start half an exp early
            nc.scalar.activation(ex[:, 0, :], scp[:, 0, :], Exp)
            nc.scalar.activation(ex[:, 1, :], scp[:, 1, :], Exp)
        else:
            nc.scalar.activation(ex[:], scp[:], Exp)
        ex_store[(pi, si)] = ex

    def attnv(pi, si):
        tb, p = PAIRS[pi]
        ensure(("v", si // 2))
        if si == 0:
            atp_store[pi] = atps.tile([128, 8, 128], f32,
                                      name=f"{R}atp{pi}", tag="at")
        atp = atp_store[pi]
        ex = ex_store.pop((pi, si))
        for h in range(2):
            for tcn in range(TCN):
                u = h * TCN + tcn
                # start=True zeroes the whole bank on HW: chains u=0..3
                # live in bank A (zeroed by u==0), u=4..7 in bank B
                # (zeroed by u==4); all siblings accumulate.
                nc.tensor.matmul(
                    atp[:, u, 0:65],
                    ex[:, h, tcn * 128:(tcn + 1) * 128],
                    v_aug[:, si, p * 2 + h, 0:65],
                    start=(si == 0 and u % 4 == 0), stop=(si == NS - 1),
                    skip_group_check=True)

    def normalize(pi):
        from concourse.bass import broadcast_tensor_aps
        tb, p = PAIRS[pi]
        atp = atp_store.pop(pi)
        a2n = a2pool.tile([128, TCN, 128], bf16, name=f"{R}a2n{pi}",
                          tag="a2n")
        nc.vector.reciprocal(rec8[:], atp[:, :, 64:65])
        # all 8 (h, tc) slots normalized in ONE DVE op: the reciprocal
        # column broadcasts over e via a stride-0 AP
        av = atp[:, :, 0:64].rearrange("p (h c) e -> p c h e", h=2)
        rv = rec8[:].rearrange("p (h c) o -> p c h o", h=2)
        av2, rv2 = broadcast_tensor_aps(av, rv)
        nc.vector.tensor_tensor(
            a2n[:].rearrange("p c (h e) -> p c h e", h=2), av2, rv2,
            op=Mult)
        a2n_store[pi] = a2n

    def dma_transpose(pi):
        tb, p = PAIRS[pi]
        a2n = a2n_store.pop(pi)
        for tcn in range(TCN):
            nc.sync.dma_start_transpose(
                at2[p][:, tb * TB + tcn * 128: tb * TB + (tcn + 1) * 128],
                a2n[:, tcn, :])

    # ---- main pipeline ----
    for pi in range(len(PAIRS)):
        for si in range(NS):
            gs = pi * NS + si
            CUR_STEP[0] = gs
            scores_exp(pi, si)
            if pi == len(PAIRS) - 1:
                # last pair: drain the previous pair's attnV at double
                # rate, normalize it mid-pair, then chase this pair's own
                # attnV so the tail is short.
                if si < 8:
                    attnv(pi - 1, 2 * si)
                    attnv(pi - 1, 2 * si + 1)
                    budget = 220
                elif si == 8:
                    normalize(pi - 1)
                    dma_transpose(pi - 1)
                    # half of the second-to-last tb's out_proj runs here;
                    # the other half fills the tail's dead PE time
                    tb_p = PAIRS[pi - 1][0]
                    for ts in range(2):
                        filler_item(gs + 1, out_unit(tb_p, ts))
                    budget = 300
                else:  # si 9..15: emit attnV(pi) for si 0..2*(si-9)+1
                    attnv(pi, 2 * (si - 9))
                    attnv(pi, 2 * (si - 9) + 1)
                    if si == NS - 1:
                        # exp(pi,14) is already done by now: chase one more
                        attnv(pi, 14)
                    budget = 300
            elif pi > 0:
                if si < 2:
                    # pair start: the deferred attnV waits on the previous
                    # pair's normalize (atp WAR); emit fillers first so
                    # the PE queue head is not blocked on it
                    run_fillers(gs, 380)
                    attnv(pi - 1, si)
                    budget = 0
                else:
                    attnv(pi - 1, si)
                    budget = 380
            else:
                budget = 650
            run_fillers(gs, budget)
        if 0 < pi < len(PAIRS) - 1:
            normalize(pi - 1)
            dma_transpose(pi - 1)
            if PAIRS[pi - 1][1] == 1:
                # at2 for this tb is now complete on both pairs ->
                # out-projection becomes available filler work
                tb = PAIRS[pi - 1][0]
                for ts in range(TCN):
                    filler_item(pi * NS + 1, out_unit(tb, ts))

    # ---- tail: finish last pair per t-chunk, ACT helps with drains ----
    Iden = mybir.ActivationFunctionType.Identity
    last = len(PAIRS) - 1
    tb3 = PAIRS[last][0]
    attnv(last, NS - 1)
    run_fillers(10 ** 9, 10 ** 9)  # stragglers
    from concourse.bass import broadcast_tensor_aps
    atp = atp_store.pop(last)
    a2n = a2pool.tile([128, TCN, 128], bf16, name=f"{R}a2nT", tag="a2n")
    nc.vector.reciprocal(rec8[:], atp[:, :, 64:65])
    av = atp[:, :, 0:64].rearrange("p (h c) e -> p c h e", h=2)
    rv = rec8[:].rearrange("p (h c) o -> p c h o", h=2)
    av2, rv2 = broadcast_tensor_aps(av, rv)
    nc.vector.tensor_tensor(
        a2n[:].rearrange("p c (h e) -> p c h e", h=2), av2, rv2, op=Mult)
    # PE-transpose into psum (53ns each) + copy instead of DMA-transpose:
    # saves the ~2.3us DGE/sem latency on the tail critical path.
    # Phase-ordered emission (all transposes -> all copies -> fills with
    # drains/DMAs chasing) so the in-order PE queue never interleaves a
    # stalled op ahead of ready fills.
    tp = atps.tile([128, TCN, 128], bf16, name=f"{R}tpT", tag="at")
    for tcn in range(TCN):
        nc.tensor.matmul(tp[:, tcn, :], a2n[:, tcn, :], id_t[:],
                         is_transpose=True, start=(tcn == 0), stop=True,
                         skip_group_check=True)
    for tcn in range(TCN):
        at2s = at2[1][:, tb3 * TB + tcn * 128: tb3 * TB + (tcn + 1) * 128]
        if tcn % 2 == 0:
            nc.vector.tensor_copy(at2s, tp[:, tcn, :])
        else:
            nc.scalar.activation(at2s, tp[:, tcn, :], Iden)
    # the deferred half of out(tb2) fills the PE while the at2 copies'
    # semaphores propagate
    for ts in (2, 3):
        for _, fn in out_unit(PAIRS[-2][0], ts):
            fn()
    psds, osbs = [], []
    for tcn in range(TCN):
        if tcn % 2 == 0:
            psd = [opps.tile([128, TB], f32, name=f"{R}ot{tcn}_{dc}",
                             tag=next_tag()) for dc in range(2)]
        else:
            scpair = scps.tile([128, 2, TB], f32, name=f"{R}ot{tcn}",
                               tag="sc")
            psd = [scpair[:, 0, :], scpair[:, 1, :]]
        psds.append(psd)
        osbs.append(ospool.tile([128, D], bf16, name=f"{R}ot{tcn}sb",
                                tag="os"))
    for tcn in range(TCN):
        for dc in range(2):
            for p in range(2):
                nc.tensor.matmul(
                    psds[tcn][dc][:],
                    at2[p][:, tb3 * TB + tcn * 128:
                           tb3 * TB + tcn * 128 + 128],
                    wo_t[:, p, dc * TB:(dc + 1) * TB],
                    start=(p == 0), stop=(p == 1))
            if dc == 0:
                nc.vector.tensor_copy(osbs[tcn][:, 0:TB], psds[tcn][0][:])
            else:
                nc.scalar.activation(osbs[tcn][:, TB:D], psds[tcn][1][:],
                                     Iden)
        nc.sync.dma_start(
            out_d[tb3 * TB + tcn * 128: tb3 * TB + (tcn + 1) * 128, :],
            osbs[tcn][:])

    octx.close()


def _get_program(reps=1):
    global _PROGRAM
    if _PROGRAM is None:
        _PROGRAM = {}
    if reps not in _PROGRAM:
        _PROGRAM[reps] = _build_program(reps)
    return _PROGRAM[reps]


def _shard_inputs(inputs):
    """Build the 8 per-core input maps from the full-problem inputs."""
    bf16 = ml_dtypes.bfloat16
    hs = np.asarray(inputs["hidden_states"], np.float32)
    pe = np.asarray(inputs["position_embeddings"], np.float32)
    Wq = np.asarray(inputs["Wq"], np.float32).reshape(D, H * HD)
    Wk = np.asarray(inputs["Wk"], np.float32).reshape(D, H * HD)
    Wv = np.asarray(inputs["Wv"], np.float32).reshape(D, H * HD)
    Wo = np.asarray(inputs["Wo"], np.float32)
    bq = np.asarray(inputs["bq"], np.float32).reshape(H * HD)
    bk = np.asarray(inputs["bk"], np.float32).reshape(H * HD)
    bv = np.asarray(inputs["bv"], np.float32).reshape(H * HD)

    h = hs + pe
    hT = [np.ascontiguousarray(h[b].T).astype(bf16) for b in range(B)]
    xT = [np.ascontiguousarray(hs[b].T).astype(bf16) for b in range(B)]

    in_maps = []
    for c in range(8):
        b, g = divmod(c, G)
        sel = slice(g * E, (g + 1) * E)
        in_maps.append({
            "hT": hT[b],
            "xT": xT[b],
            "wq": (np.ascontiguousarray(Wq[:, sel])
                   * np.float32(SCALE)).astype(bf16),
            "wk": np.ascontiguousarray(Wk[:, sel]).astype(bf16),
            "wv": np.ascontiguousarray(Wv[:, sel]).astype(bf16),
            "wo": np.ascontiguousarray(Wo[sel, :]).astype(bf16),
            "bq": (bq[sel] * np.float32(SCALE)).reshape(2, 128, 1).copy(),
            "bk": bk[sel].reshape(2, 128, 1).copy(),
            "bvr": np.tile(bv[sel][None, :], (128, 1)),
        })
    return in_maps


def _gather_outputs(results, inputs):
    bo = np.asarray(inputs["bo"], np.float32)
    out = np.empty((B, S, D), np.float32)
    for b in range(B):
        acc = results[4 * b]["out"].astype(np.float32).copy()
        for g in range(1, G):
            acc += results[4 * b + g]["out"]
        out[b] = acc + bo[None, :]
    return out


def kernel(**inputs):
    from concourse.bass_utils import run_bass_kernel_spmd

    nc = _get_program()
    in_maps = _shard_inputs(inputs)
    res = run_bass_kernel_spmd(nc, in_maps, list(range(8)))
    return _gather_outputs(res.results, inputs)


# revision 45
# speedup vs baseline: 1.5531x; 1.0113x over previous
"""Trainium2 Bass kernel for DFine multi-head attention (v2, bf16).

Problem: B=2, S=2048, D=1024, H=16 heads, HD=64.
Sharding over 8 cores: core c handles batch b=c//4 and head-group g=c%4
(4 heads). Each core computes its heads' attention and a partial
out-projection [2048, 1024]; the host sums the 4 partials per batch and
adds the output bias.

v2 design (vs fp32r baseline):
- All matmul operands bf16 (1 cyc/row at any moving size); psum f32.
- attnV swapped: stationary = exp-tile [128s x 128t], moving = v [128s, 65]
  (64 + ones column for the softmax denominator): 65-row matmuls instead of
  512-row ones -> halves attnV PE rows.
- attnV output lands [t, head_e] in psum, so the denominator is a
  per-partition scalar: reciprocal + tensor_scalar normalize, then a
  DMA transpose (xbar) produces the [e, t] layout for the out-projection.
- out-projection DMAs straight from PSUM to DRAM (no SBUF staging).
- h = x + pos precomputed on host; inputs DMAd bf16 (half the bytes).
- Static software pipeline: per si-step emit scores -> exp -> deferred
  attnV (one pair behind, so v/atp dependencies are off the critical
  path) -> projection/out_proj filler matmuls from a deadline queue.
"""

import sys
import numpy as np
import ml_dtypes

if "/opt/trn_rl_repo" not in sys.path:
    sys.path.insert(0, "/opt/trn_rl_repo")

B, S, D, H, HD = 2, 2048, 1024, 16, 64
G = 4          # heads per core
E = G * HD     # 256 per-core head width
T = S
KC = 8         # contraction chunks of 128 over D
TB = 512       # t-block
NT = T // TB   # 4
NS = T // 128  # 16 s-chunks
TCN = TB // 128  # 4 t-chunks per t-block
SCALE = HD ** -0.5

# pair order: all p=0 pairs first so kT/qT for p=1 and the second half of
# the projection work is not demanded in the first two pairs.
PAIRS = [(0, 0), (1, 0), (2, 0), (3, 0), (0, 1), (1, 1), (2, 1), (3, 1)]

_PROGRAM = None
_DBG = False


def _build_program(reps=1):
    import concourse.bacc as bacc
    import concourse.tile as tile
    from concourse import mybir

    f32 = mybir.dt.float32
    bf16 = mybir.dt.bfloat16

    nc = bacc.Bacc("TRN2", target_bir_lowering=False, debug=False)

    hT_d = nc.declare_dram_parameter("hT", [D, T], bf16, isOutput=False)
    xT_d = nc.declare_dram_parameter("xT", [D, T], bf16, isOutput=False)
    wq_d = nc.declare_dram_parameter("wq", [D, E], bf16, isOutput=False)
    wk_d = nc.declare_dram_parameter("wk", [D, E], bf16, isOutput=False)
    wv_d = nc.declare_dram_parameter("wv", [D, E], bf16, isOutput=False)
    wo_d = nc.declare_dram_parameter("wo", [E, D], bf16, isOutput=False)
    bq_d = nc.declare_dram_parameter("bq", [2, 128, 1], f32, isOutput=False)
    bk_d = nc.declare_dram_parameter("bk", [2, 128, 1], f32, isOutput=False)
    bv_d = nc.declare_dram_parameter("bvr", [128, E], f32, isOutput=False)
    out_d = nc.declare_dram_parameter("out", [T, D], bf16, isOutput=True)

    with tile.TileContext(nc) as tc:
        for rep in range(reps):
            _build_body(nc, tc, mybir, rep,
                        (hT_d, xT_d, wq_d, wk_d, wv_d, wo_d, bq_d, bk_d,
                         bv_d, out_d))

    nc.compile()
    return nc


def _build_body(nc, tc, mybir, rep, drams):
    from contextlib import ExitStack

    f32 = mybir.dt.float32
    bf16 = mybir.dt.bfloat16
    Exp = mybir.ActivationFunctionType.Exp
    Add = mybir.AluOpType.add
    Mult = mybir.AluOpType.mult
    (hT_d, xT_d, wq_d, wk_d, wv_d, wo_d, bq_d, bk_d, bv_d, out_d) = drams
    R = f"r{rep}_"

    octx = ExitStack()
    wpool = octx.enter_context(tc.tile_pool(name=f"{R}wpool", bufs=1))
    expool = octx.enter_context(tc.tile_pool(name=f"{R}expool", bufs=18))
    a2pool = octx.enter_context(tc.tile_pool(name=f"{R}a2pool", bufs=2))
    ospool = octx.enter_context(tc.tile_pool(name=f"{R}ospool", bufs=4))
    scps = octx.enter_context(tc.tile_pool(name=f"{R}scps", bufs=2,
                                           space="PSUM"))
    atps = octx.enter_context(tc.tile_pool(name=f"{R}atps", bufs=1,
                                           space="PSUM"))
    opps = octx.enter_context(tc.tile_pool(name=f"{R}opps", bufs=1,
                                           space="PSUM"))

    # ---- persistent SBUF tiles ----
    wq_t = wpool.tile([128, KC, E], bf16, name=f"{R}wq_t")
    wk_t = wpool.tile([128, KC, E], bf16, name=f"{R}wk_t")
    wv_t = wpool.tile([128, KC, E], bf16, name=f"{R}wv_t")
    wo_t = wpool.tile([128, 2, D], bf16, name=f"{R}wo_t")
    bq_t = wpool.tile([128, 2, 1], f32, name=f"{R}bq_t")
    bk_t = wpool.tile([128, 2, 1], f32, name=f"{R}bk_t")
    bv_t = wpool.tile([128, E], f32, name=f"{R}bv_t")
    hT_t = wpool.tile([128, KC, T], bf16, name=f"{R}hT_t")
    xT_t = wpool.tile([128, KC, T], bf16, name=f"{R}xT_t")
    qT = [wpool.tile([128, T], bf16, name=f"{R}qT{p}") for p in range(2)]
    kT = [wpool.tile([128, T], bf16, name=f"{R}kT{p}") for p in range(2)]
    v_aug = wpool.tile([128, NS, G, 66], bf16, name=f"{R}v_aug")
    at2 = [wpool.tile([128, T], bf16, name=f"{R}at2_{p}") for p in range(2)]
    rec8 = wpool.tile([128, 8, 1], f32, name=f"{R}rec8")
    onecol = wpool.tile([128, NS, G, 1], bf16, name=f"{R}onecol")
    id_t = wpool.tile([128, 128], bf16, name=f"{R}id_t")

    from concourse import masks
    masks.make_identity(nc, id_t[:])
    nc.gpsimd.memset(onecol[:], 1.0)
    nc.gpsimd.tensor_copy(v_aug[:, :, :, 64:65], onecol[:])
    # dummy exp at t=0 so the 1.3us activation-table load happens under
    # the input DMAs instead of right before the first real exp
    warm = wpool.tile([1, 1], f32, name=f"{R}warm")
    nc.scalar.activation(warm[:], warm[:], Exp)
    # PE p-state warm-up: ~3us of dummy matmuls so the tensor engine is
    # at full clock when the first projection chunk lands
    wps = opps.tile([128, 128], f32, name=f"{R}wps", tag="op0")
    for i in range(26):
        nc.tensor.matmul(wps[:], id_t[:], id_t[:], start=(i == 0),
                         stop=(i == 25), skip_group_check=True)

    # ---- DMA emission (SP queue, FIFO) ----
    # wk, wq first; then hT t-block 0 chunk-by-chunk with the first k/q
    # projection matmuls chasing each chunk so scores can start ~9us in.
    # DMA order tuned for the first-scores critical path: wk, two hT
    # chunks (k-matmuls start ramping the PE), then wq, the rest of the
    # chunks, and the (tiny) biases last
    nc.sync.dma_start(
        wk_t[:], wk_d[:].rearrange("(c p) e -> p c e", p=128))
    ps_k0 = opps.tile([128, TB], f32, name=f"{R}k0s0ps", tag="op0")
    ps_q0 = opps.tile([128, TB], f32, name=f"{R}q0t0ps", tag="op1")
    kmm = [lambda k=k: nc.tensor.matmul(
        ps_k0[:], wk_t[:, k, 0:128], hT_t[:, k, 0:TB],
        start=(k == 0), stop=(k == KC - 1)) for k in range(KC)]
    qmm = [lambda k=k: nc.tensor.matmul(
        ps_q0[:], wq_t[:, k, 0:128], hT_t[:, k, 0:TB],
        start=(k == 0), stop=(k == KC - 1)) for k in range(KC)]
    for k in range(2):
        nc.sync.dma_start(hT_t[:, k, 0:TB], hT_d[k * 128:(k + 1) * 128, 0:TB])
        kmm[k]()
    nc.sync.dma_start(
        wq_t[:], wq_d[:].rearrange("(c p) e -> p c e", p=128))
    for k in range(2, KC):
        nc.sync.dma_start(hT_t[:, k, 0:TB], hT_d[k * 128:(k + 1) * 128, 0:TB])
        kmm[k]()
        qmm[k - 2]()
    qmm[KC - 2]()
    qmm[KC - 1]()
    nc.sync.dma_start(bk_t[:], bk_d[:].rearrange("c p o -> p c o"))
    nc.sync.dma_start(bq_t[:], bq_d[:].rearrange("c p o -> p c o"))
    nc.sync.dma_start(bv_t[:], bv_d[:])
    # k-drain on DVE, q-drain on the (idle at startup) ACT engine so the
    # two don't serialize ahead of the first scores
    nc.vector.tensor_scalar(kT[0][:, 0:TB], ps_k0[:], bk_t[:, 0, :], None,
                            Add)
    nc.scalar.activation(qT[0][:, 0:TB], ps_q0[:],
                         mybir.ActivationFunctionType.Identity,
                         bias=bq_t[:, 0, :])

    def _hq(qd):
        nc.sync.dma_start(
            hT_t[:, :, qd * TB:(qd + 1) * TB],
            hT_d[:, qd * TB:(qd + 1) * TB].rearrange("(c p) t -> p c t",
                                                     p=128))

    def _xq(qd):
        nc.sync.dma_start(
            xT_t[:, :, qd * TB:(qd + 1) * TB],
            xT_d[:, qd * TB:(qd + 1) * TB].rearrange("(c p) t -> p c t",
                                                     p=128))

    _hq(1)
    _hq(2)
    _hq(3)
    nc.sync.dma_start(
        wv_t[:], wv_d[:].rearrange("(c p) e -> p c e", p=128))
    _xq(0)
    _xq(1)
    _xq(2)
    _xq(3)
    nc.sync.dma_start(
        wo_t[:], wo_d[:].rearrange("(c p) d -> p c d", p=128))

    # ---- filler queue: deadline-ordered projection / out_proj work ----
    tag_i = [0]

    def next_tag():
        t = f"op{tag_i[0] % 2}"
        tag_i[0] += 1
        return t

    def qk_group(w_t, b_t, dstT, p, blk, nm):
        box = {}
        tag = [None]

        def mk_mm(k):
            def f():
                if k == 0:
                    tag[0] = next_tag()
                    box["ps"] = opps.tile([128, TB], f32,
                                          name=f"{R}{nm}ps", tag=tag[0])
                nc.tensor.matmul(box["ps"][:],
                                 w_t[:, k, p * 128:(p + 1) * 128],
                                 hT_t[:, k, blk * TB:(blk + 1) * TB],
                                 start=(k == 0), stop=(k == KC - 1))
            return f

        ops = [(213, mk_mm(k)) for k in range(KC)]

        def drain():
            nc.vector.tensor_scalar(dstT[p][:, blk * TB:(blk + 1) * TB],
                                    box["ps"][:], b_t[:, p, :], None, Add)
        ops.append((0, drain))
        return ops

    def v_group(j):
        # si pair (2j, 2j+1): two 8-matmul chains into one psum bank
        box = {}
        tag = [None]

        def mk_mm(k, jj):
            def f():
                if k == 0 and jj == 0:
                    tag[0] = next_tag()
                    box["ps"] = opps.tile([128, 2, E], f32,
                                          name=f"{R}v{j}ps", tag=tag[0])
                si = 2 * j + jj
                # HW: start=True zeroes the whole psum bank, so only the
                # first chain in the bank starts; the sibling accumulates.
                nc.tensor.matmul(box["ps"][:, jj, :],
                                 xT_t[:, k, si * 128:(si + 1) * 128],
                                 wv_t[:, k, :],
                                 start=(k == 0 and jj == 0),
                                 stop=(k == KC - 1),
                                 skip_group_check=True)
            return f

        ops = []
        for k in range(KC):
            for jj in range(2):
                ops.append((107, mk_mm(k, jj)))

        def mk_drain(jj):
            def f():
                si = 2 * j + jj
                nc.vector.tensor_tensor(
                    v_aug[:, si, :, 0:64],
                    box["ps"][:, jj, :].rearrange("p (g e) -> p g e", g=G),
                    bv_t[:].rearrange("p (g e) -> p g e", g=G),
                    op=Add)
            return f
        ops.append((0, mk_drain(0)))
        ops.append((0, mk_drain(1)))
        return ops

    def out_unit(tb, ts):
        # one 128-row output slice: dc0 fills one psum bank, dc1 the
        # other; both drain into one bf16 staging row -> single DMA.
        box = {}

        def mk_mm(dc, p):
            def f():
                if p == 0:
                    box[dc] = opps.tile([128, TB], f32,
                                        name=f"{R}o{tb}_{ts}_{dc}ps",
                                        tag=next_tag())
                nc.tensor.matmul(box[dc][:],
                                 at2[p][:, tb * TB + ts * 128:
                                        tb * TB + ts * 128 + 128],
                                 wo_t[:, p, dc * TB:(dc + 1) * TB],
                                 start=(p == 0), stop=(p == 1))
            return f

        def mk_drain(dc):
            def f():
                if dc == 0:
                    box["osb"] = ospool.tile([128, D], bf16,
                                             name=f"{R}o{tb}_{ts}sb",
                                             tag="os")
                nc.vector.tensor_copy(
                    box["osb"][:, dc * TB:(dc + 1) * TB], box[dc][:])
            return f

        def dma():
            nc.sync.dma_start(
                out_d[tb * TB + ts * 128: tb * TB + (ts + 1) * 128, :],
                box["osb"][:])
        return [(213, mk_mm(0, 0)), (213, mk_mm(0, 1)), (0, mk_drain(0)),
                (213, mk_mm(1, 0)), (213, mk_mm(1, 1)), (0, mk_drain(1)),
                (0, dma)]

    fillers = []
    filler_by_key = {}

    def filler_item(ready, ops, key=None):
        it = {"ready": ready, "ops": ops, "i": 0, "key": key}
        fillers.append(it)
        if key is not None:
            filler_by_key[key] = it

    def ensure(key, _dbg=[0]):
        """Force-emit every remaining op of the filler item `key` so a
        consumer emitted next observes its writes (tile deps only order
        instructions that are already emitted)."""
        it = filler_by_key.get(key)
        if it is None:
            return
        n = len(it["ops"]) - it["i"]
        if n > 0 and _DBG:
            print(f"ENSURE {key} forces {n} ops at step {CUR_STEP[0]}")
        while it["i"] < len(it["ops"]):
            _, fn = it["ops"][it["i"]]
            it["i"] += 1
            fn()

    # deadline-ordered: (the scan picks the first *ready* item)
    filler_item(1, qk_group(wk_t, bk_t, kT, 0, 1, "k0s1"), ("k", 0, 1))
    filler_item(4, qk_group(wk_t, bk_t, kT, 0, 2, "k0s2"), ("k", 0, 2))
    filler_item(7, qk_group(wk_t, bk_t, kT, 0, 3, "k0s3"), ("k", 0, 3))
    filler_item(2, qk_group(wq_t, bq_t, qT, 0, 1, "q0t1"), ("q", 0, 1))
    filler_item(4, qk_group(wq_t, bq_t, qT, 0, 2, "q0t2"), ("q", 0, 2))
    filler_item(11, v_group(0), ("v", 0))
    filler_item(11, v_group(1), ("v", 1))
    filler_item(14, v_group(2), ("v", 2))
    filler_item(14, v_group(3), ("v", 3))
    filler_item(17, v_group(4), ("v", 4))
    filler_item(17, v_group(5), ("v", 5))
    filler_item(20, v_group(6), ("v", 6))
    filler_item(20, v_group(7), ("v", 7))
    filler_item(7, qk_group(wq_t, bq_t, qT, 0, 3, "q0t3"), ("q", 0, 3))
    filler_item(28, qk_group(wk_t, bk_t, kT, 1, 0, "k1s0"), ("k", 1, 0))
    filler_item(31, qk_group(wq_t, bq_t, qT, 1, 0, "q1t0"), ("q", 1, 0))
    filler_item(40, qk_group(wk_t, bk_t, kT, 1, 1, "k1s1"), ("k", 1, 1))
    filler_item(48, qk_group(wk_t, bk_t, kT, 1, 2, "k1s2"), ("k", 1, 2))
    filler_item(56, qk_group(wk_t, bk_t, kT, 1, 3, "k1s3"), ("k", 1, 3))
    filler_item(60, qk_group(wq_t, bq_t, qT, 1, 1, "q1t1"), ("q", 1, 1))
    filler_item(64, qk_group(wq_t, bq_t, qT, 1, 2, "q1t2"), ("q", 1, 2))
    filler_item(68, qk_group(wq_t, bq_t, qT, 1, 3, "q1t3"), ("q", 1, 3))

    def run_fillers(step, budget):
        spent = 0
        while spent < budget:
            it = None
            for x in fillers:
                if x["i"] < len(x["ops"]) and x["ready"] <= step:
                    it = x
                    break
            if it is None:
                return
            cost, fn = it["ops"][it["i"]]
            it["i"] += 1
            fn()
            spent += cost

    # ---- attention machinery ----
    CUR_STEP = [0]
    ex_store = {}
    atp_store = {}
    a2n_store = {}

    def scores_exp(pi, si):
        tb, p = PAIRS[pi]
        ensure(("k", p, si // 4))
        ensure(("q", p, tb))
        scp = scps.tile([128, 2, TB], f32, name=f"{R}sc{pi}_{si}", tag="sc")
        for h in range(2):
            nc.tensor.matmul(
                scp[:, h, :],
                kT[p][h * 64:(h + 1) * 64, si * 128:(si + 1) * 128],
                qT[p][h * 64:(h + 1) * 64, tb * TB:(tb + 1) * TB],
                start=True, stop=True)
        ex = expool.tile([128, 2, TB], bf16, name=f"{R}ex{pi}_{si}", tag="ex")
        if pi == len(PAIRS) - 1 and si == NS - 1:
            # the very last exp gates the tail: split by head so the h0
            # attnV chains (and everything after) start half an exp early
            nc.scalar.activation(ex[:, 0, :], scp[:, 0, :], Exp)
            nc.scalar.activation(ex[:, 1, :], scp[:, 1, :], Exp)
        else:
            nc.scalar.activation(ex[:], scp[:], Exp)
        ex_store[(pi, si)] = ex

    def attnv(pi, si):
        tb, p = PAIRS[pi]
        ensure(("v", si // 2))
        if si == 0:
            atp_store[pi] = atps.tile([128, 8, 128], f32,
                                      name=f"{R}atp{pi}", tag="at")
        atp = atp_store[pi]
        ex = ex_store.pop((pi, si))
        for h in range(2):
            for tcn in range(TCN):
                u = h * TCN + tcn
                # start=True zeroes the whole bank on HW: chains u=0..3
                # live in bank A (zeroed by u==0), u=4..7 in bank B
                # (zeroed by u==4); all siblings accumulate.
                nc.tensor.matmul(
                    atp[:, u, 0:65],
                    ex[:, h, tcn * 128:(tcn + 1) * 128],
                    v_aug[:, si, p * 2 + h, 0:65],
                    start=(si == 0 and u % 4 == 0), stop=(si == NS - 1),
                    skip_group_check=True)

    def normalize(pi):
        from concourse.bass import broadcast_tensor_aps
        tb, p = PAIRS[pi]
        atp = atp_store.pop(pi)
        a2n = a2pool.tile([128, TCN, 128], bf16, name=f"{R}a2n{pi}",
                          tag="a2n")
        nc.vector.reciprocal(rec8[:], atp[:, :, 64:65])
        # all 8 (h, tc) slots normalized in ONE DVE op: the reciprocal
        # column broadcasts over e via a stride-0 AP
        av = atp[:, :, 0:64].rearrange("p (h c) e -> p c h e", h=2)
        rv = rec8[:].rearrange("p (h c) o -> p c h o", h=2)
        av2, rv2 = broadcast_tensor_aps(av, rv)
        nc.vector.tensor_tensor(
            a2n[:].rearrange("p c (h e) -> p c h e", h=2), av2, rv2,
            op=Mult)
        a2n_store[pi] = a2n

    def dma_transpose(pi):
        tb, p = PAIRS[pi]
        a2n = a2n_store.pop(pi)
        for tcn in range(TCN):
            nc.sync.dma_start_transpose(
                at2[p][:, tb * TB + tcn * 128: tb * TB + (tcn + 1) * 128],
                a2n[:, tcn, :])

    # ---- main pipeline ----
    for pi in range(len(PAIRS)):
        for si in range(NS):
            gs = pi * NS + si
            CUR_STEP[0] = gs
            scores_exp(pi, si)
            # deferred attnV runs one step late within the pair (chains
            # have a whole pair of slack), so the pair-boundary atp WAR
            # on the previous normalize resolves during step 0
            if pi == len(PAIRS) - 1:
                # last pair: drain the previous pair's attnV at double
                # rate, normalize it mid-pair, then chase this pair's own
                # attnV so the tail is short.
                if si == 0:
                    budget = 380
                elif si <= 8:
                    attnv(pi - 1, 2 * (si - 1))
                    attnv(pi - 1, 2 * si - 1)
                    budget = 220
                    if si == 8:
                        normalize(pi - 1)
                        dma_transpose(pi - 1)
                        # half of the second-to-last tb's out_proj runs
                        # here; the rest fills the tail's dead PE time
                        tb_p = PAIRS[pi - 1][0]
                        for ts in range(2):
                            filler_item(gs + 1, out_unit(tb_p, ts))
                        budget = 300
                else:  # si 9..15: emit attnV(pi) for si 0..2*(si-9)+1
                    attnv(pi, 2 * (si - 9))
                    attnv(pi, 2 * (si - 9) + 1)
                    if si == NS - 1:
                        # exp(pi,14) is already done by now: chase one more
                        attnv(pi, 14)
                    budget = 300
            elif pi > 0:
                if si == 0:
                    budget = 380
                elif si == NS - 1:
                    attnv(pi - 1, si - 1)
                    attnv(pi - 1, si)
                    budget = 300
                else:
                    attnv(pi - 1, si - 1)
                    budget = 380
            else:
                budget = 650
            run_fillers(gs, budget)
        if 0 < pi < len(PAIRS) - 1:
            normalize(pi - 1)
            dma_transpose(pi - 1)
            if PAIRS[pi - 1][1] == 1:
                # at2 for this tb is now complete on both pairs ->
                # out-projection becomes available filler work
                tb = PAIRS[pi - 1][0]
                for ts in range(TCN):
                    filler_item(pi * NS + 1, out_unit(tb, ts))

    # ---- tail: finish last pair per t-chunk, ACT helps with drains ----
    Iden = mybir.ActivationFunctionType.Identity
    last = len(PAIRS) - 1
    tb3 = PAIRS[last][0]
    attnv(last, NS - 1)
    run_fillers(10 ** 9, 10 ** 9)  # stragglers
    # the deferred half of out(tb2): its fills overlap the normalize
    for ts in (2, 3):
        for _, fn in out_unit(PAIRS[-2][0], ts):
            fn()
    from concourse.bass import broadcast_tensor_aps
    atp = atp_store.pop(last)
    a2n = a2pool.tile([128, TCN, 128], bf16, name=f"{R}a2nT", tag="a2n")
    nc.vector.reciprocal(rec8[:], atp[:, :, 64:65])
    av = atp[:, :, 0:64].rearrange("p (h c) e -> p c h e", h=2)
    rv = rec8[:].rearrange("p (h c) o -> p c h o", h=2)
    av2, rv2 = broadcast_tensor_aps(av, rv)
    nc.vector.tensor_tensor(
        a2n[:].rearrange("p c (h e) -> p c h e", h=2), av2, rv2, op=Mult)
    # PE-transpose into psum (53ns each) + copy instead of DMA-transpose:
    # saves the ~2.3us DGE/sem latency on the tail critical path.
    # Phase-ordered emission (all transposes -> all copies -> fills with
    # drains/DMAs chasing) so the in-order PE queue never interleaves a
    # stalled op ahead of ready fills.
    tp = atps.tile([128, TCN, 128], bf16, name=f"{R}tpT", tag="at")
    for tcn in range(TCN):
        nc.tensor.matmul(tp[:, tcn, :], a2n[:, tcn, :], id_t[:],
                         is_transpose=True, start=(tcn == 0), stop=True,
                         skip_group_check=True)
    for tcn in range(TCN):
        at2s = at2[1][:, tb3 * TB + tcn * 128: tb3 * TB + (tcn + 1) * 128]
        if tcn % 2 == 0:
            nc.vector.tensor_copy(at2s, tp[:, tcn, :])
        else:
            nc.scalar.activation(at2s, tp[:, tcn, :], Iden)
    psds, osbs = [], []
    for tcn in range(TCN):
        if tcn % 2 == 0:
            psd = [opps.tile([128, TB], f32, name=f"{R}ot{tcn}_{dc}",
                             tag=next_tag()) for dc in range(2)]
        else:
            scpair = scps.tile([128, 2, TB], f32, name=f"{R}ot{tcn}",
                               tag="sc")
            psd = [scpair[:, 0, :], scpair[:, 1, :]]
        psds.append(psd)
        osbs.append(ospool.tile([128, D], bf16, name=f"{R}ot{tcn}sb",
                                tag="os"))
    for tcn in range(TCN):
        for dc in range(2):
            for p in range(2):
                nc.tensor.matmul(
                    psds[tcn][dc][:],
                    at2[p][:, tb3 * TB + tcn * 128:
                           tb3 * TB + tcn * 128 + 128],
                    wo_t[:, p, dc * TB:(dc + 1) * TB],
                    start=(p == 0), stop=(p == 1))
            if dc == 0:
                nc.vector.tensor_copy(osbs[tcn][:, 0:TB], psds[tcn][0][:])
            else:
                nc.scalar.activation(osbs[tcn][:, TB:D], psds[tcn][1][:],
                                     Iden)
        nc.sync.dma_start(
            out_d[tb3 * TB + tcn * 128: tb3 * TB + (tcn + 1) * 128, :],
            osbs[tcn][:])

    octx.close()


def _get_program(reps=1):
    global _PROGRAM
    if _PROGRAM is None:
        _PROGRAM = {}
    if reps not in _PROGRAM:
        _PROGRAM[reps] = _build_program(reps)
    return _PROGRAM[reps]


def _shard_inputs(inputs):
    """Build the 8 per-core input maps from the full-problem inputs."""
    bf16 = ml_dtypes.bfloat16
    hs = np.asarray(inputs["hidden_states"], np.float32)
    pe = np.asarray(inputs["position_embeddings"], np.float32)
    Wq = np.asarray(inputs["Wq"], np.float32).reshape(D, H * HD)
    Wk = np.asarray(inputs["Wk"], np.float32).reshape(D, H * HD)
    Wv = np.asarray(inputs["Wv"], np.float32).reshape(D, H * HD)
    Wo = np.asarray(inputs["Wo"], np.float32)
    bq = np.asarray(inputs["bq"], np.float32).reshape(H * HD)
    bk = np.asarray(inputs["bk"], np.float32).reshape(H * HD)
    bv = np.asarray(inputs["bv"], np.float32).reshape(H * HD)

    h = hs + pe
    hT = [np.ascontiguousarray(h[b].T).astype(bf16) for b in range(B)]
    xT = [np.ascontiguousarray(hs[b].T).astype(bf16) for b in range(B)]

    in_maps = []
    for c in range(8):
        b, g = divmod(c, G)
        sel = slice(g * E, (g + 1) * E)
        in_maps.append({
            "hT": hT[b],
            "xT": xT[b],
            "wq": (np.ascontiguousarray(Wq[:, sel])
                   * np.float32(SCALE)).astype(bf16),
            "wk": np.ascontiguousarray(Wk[:, sel]).astype(bf16),
            "wv": np.ascontiguousarray(Wv[:, sel]).astype(bf16),
            "wo": np.ascontiguousarray(Wo[sel, :]).astype(bf16),
            "bq": (bq[sel] * np.float32(SCALE)).reshape(2, 128, 1).copy(),
            "bk": bk[sel].reshape(2, 128, 1).copy(),
            "bvr": np.tile(bv[sel][None, :], (128, 1)),
        })
    return in_maps


def _gather_outputs(results, inputs):
    bo = np.asarray(inputs["bo"], np.float32)
    out = np.empty((B, S, D), np.float32)
    for b in range(B):
        acc = results[4 * b]["out"].astype(np.float32).copy()
        for g in range(1, G):
            acc += results[4 * b + g]["out"]
        out[b] = acc + bo[None, :]
    return out


def kernel(**inputs):
    from concourse.bass_utils import run_bass_kernel_spmd

    nc = _get_program()
    in_maps = _shard_inputs(inputs)
    res = run_bass_kernel_spmd(nc, in_maps, list(range(8)))
    return _gather_outputs(res.results, inputs)


# revision 58
# speedup vs baseline: 1.5539x; 1.0005x over previous
"""Trainium2 Bass kernel for DFine multi-head attention (v2, bf16).

Problem: B=2, S=2048, D=1024, H=16 heads, HD=64.
Sharding over 8 cores: core c handles batch b=c//4 and head-group g=c%4
(4 heads). Each core computes its heads' attention and a partial
out-projection [2048, 1024]; the host sums the 4 partials per batch and
adds the output bias.

v2 design (vs fp32r baseline):
- All matmul operands bf16 (1 cyc/row at any moving size); psum f32.
- attnV swapped: stationary = exp-tile [128s x 128t], moving = v [128s, 65]
  (64 + ones column for the softmax denominator): 65-row matmuls instead of
  512-row ones -> halves attnV PE rows.
- attnV output lands [t, head_e] in psum, so the denominator is a
  per-partition scalar: reciprocal + tensor_scalar normalize, then a
  DMA transpose (xbar) produces the [e, t] layout for the out-projection.
- out-projection DMAs straight from PSUM to DRAM (no SBUF staging).
- h = x + pos precomputed on host; inputs DMAd bf16 (half the bytes).
- Static software pipeline: per si-step emit scores -> exp -> deferred
  attnV (one pair behind, so v/atp dependencies are off the critical
  path) -> projection/out_proj filler matmuls from a deadline queue.
"""

import sys
import numpy as np
import ml_dtypes

if "/opt/trn_rl_repo" not in sys.path:
    sys.path.insert(0, "/opt/trn_rl_repo")

B, S, D, H, HD = 2, 2048, 1024, 16, 64
G = 4          # heads per core
E = G * HD     # 256 per-core head width
T = S
KC = 8         # contraction chunks of 128 over D
TB = 512       # t-block
NT = T // TB   # 4
NS = T // 128  # 16 s-chunks
TCN = TB // 128  # 4 t-chunks per t-block
SCALE = HD ** -0.5

# pair order: all p=0 pairs first so kT/qT for p=1 and the second half of
# the projection work is not demanded in the first two pairs.
PAIRS = [(0, 0), (1, 0), (2, 0), (3, 0), (0, 1), (1, 1), (2, 1), (3, 1)]

_PROGRAM = None
_DBG = False


def _build_program(reps=1):
    import concourse.bacc as bacc
    import concourse.tile as tile
    from concourse import mybir

    f32 = mybir.dt.float32
    bf16 = mybir.dt.bfloat16

    nc = bacc.Bacc("TRN2", target_bir_lowering=False, debug=False)

    hT_d = nc.declare_dram_parameter("hT", [D, T], bf16, isOutput=False)
    xT_d = nc.declare_dram_parameter("xT", [D, T], bf16, isOutput=False)
    wq_d = nc.declare_dram_parameter("wq", [D, E], bf16, isOutput=False)
    wk_d = nc.declare_dram_parameter("wk", [D, E], bf16, isOutput=False)
    wv_d = nc.declare_dram_parameter("wv", [D, E], bf16, isOutput=False)
    wo_d = nc.declare_dram_parameter("wo", [E, D], bf16, isOutput=False)
    bq_d = nc.declare_dram_parameter("bq", [2, 128, 1], f32, isOutput=False)
    bk_d = nc.declare_dram_parameter("bk", [2, 128, 1], f32, isOutput=False)
    bv_d = nc.declare_dram_parameter("bvr", [128, E], f32, isOutput=False)
    out_d = nc.declare_dram_parameter("out", [T, D], bf16, isOutput=True)

    with tile.TileContext(nc) as tc:
        for rep in range(reps):
            _build_body(nc, tc, mybir, rep,
                        (hT_d, xT_d, wq_d, wk_d, wv_d, wo_d, bq_d, bk_d,
                         bv_d, out_d))

    nc.compile()
    return nc


def _build_body(nc, tc, mybir, rep, drams):
    from contextlib import ExitStack

    f32 = mybir.dt.float32
    bf16 = mybir.dt.bfloat16
    Exp = mybir.ActivationFunctionType.Exp
    Add = mybir.AluOpType.add
    Mult = mybir.AluOpType.mult
    (hT_d, xT_d, wq_d, wk_d, wv_d, wo_d, bq_d, bk_d, bv_d, out_d) = drams
    R = f"r{rep}_"

    octx = ExitStack()
    wpool = octx.enter_context(tc.tile_pool(name=f"{R}wpool", bufs=1))
    expool = octx.enter_context(tc.tile_pool(name=f"{R}expool", bufs=22))
    a2pool = octx.enter_context(tc.tile_pool(name=f"{R}a2pool", bufs=2))
    ospool = octx.enter_context(tc.tile_pool(name=f"{R}ospool", bufs=4))
    scps = octx.enter_context(tc.tile_pool(name=f"{R}scps", bufs=2,
                                           space="PSUM"))
    atps = octx.enter_context(tc.tile_pool(name=f"{R}atps", bufs=1,
                                           space="PSUM"))
    opps = octx.enter_context(tc.tile_pool(name=f"{R}opps", bufs=1,
                                           space="PSUM"))

    # ---- persistent SBUF tiles ----
    wq_t = wpool.tile([128, KC, E], bf16, name=f"{R}wq_t")
    wk_t = wpool.tile([128, KC, E], bf16, name=f"{R}wk_t")
    wv_t = wpool.tile([128, KC, E], bf16, name=f"{R}wv_t")
    wo_t = wpool.tile([128, 2, D], bf16, name=f"{R}wo_t")
    bq_t = wpool.tile([128, 2, 1], f32, name=f"{R}bq_t")
    bk_t = wpool.tile([128, 2, 1], f32, name=f"{R}bk_t")
    bv_t = wpool.tile([128, E], f32, name=f"{R}bv_t")
    hT_t = wpool.tile([128, KC, T], bf16, name=f"{R}hT_t")
    xT_t = wpool.tile([128, KC, T], bf16, name=f"{R}xT_t")
    qT = [wpool.tile([128, T], bf16, name=f"{R}qT{p}") for p in range(2)]
    kT = [wpool.tile([128, T], bf16, name=f"{R}kT{p}") for p in range(2)]
    v_aug = wpool.tile([128, NS, G, 66], bf16, name=f"{R}v_aug")
    at2 = [wpool.tile([128, T], bf16, name=f"{R}at2_{p}") for p in range(2)]
    rec8 = wpool.tile([128, 8, 1], f32, name=f"{R}rec8")
    onecol = wpool.tile([128, NS, G, 1], bf16, name=f"{R}onecol")
    id_t = wpool.tile([128, 128], bf16, name=f"{R}id_t")

    from concourse import masks
    masks.make_identity(nc, id_t[:])
    nc.gpsimd.memset(onecol[:], 1.0)
    nc.gpsimd.tensor_copy(v_aug[:, :, :, 64:65], onecol[:])
    # dummy exp at t=0 so the 1.3us activation-table load happens under
    # the input DMAs instead of right before the first real exp
    warm = wpool.tile([1, 1], f32, name=f"{R}warm")
    nc.scalar.activation(warm[:], warm[:], Exp)
    # PE p-state warm-up: ~3us of dummy matmuls so the tensor engine is
    # at full clock when the first projection chunk lands
    wps = opps.tile([128, 128], f32, name=f"{R}wps", tag="op0")
    for i in range(26):
        nc.tensor.matmul(wps[:], id_t[:], id_t[:], start=(i == 0),
                         stop=(i == 25), skip_group_check=True)

    # ---- DMA emission (SP queue, FIFO) ----
    # wk, wq first; then hT t-block 0 chunk-by-chunk with the first k/q
    # projection matmuls chasing each chunk so scores can start ~9us in.
    # DMA order tuned for the first-scores critical path: wk, two hT
    # chunks (k-matmuls start ramping the PE), then wq, the rest of the
    # chunks, and the (tiny) biases last
    nc.sync.dma_start(
        wk_t[:], wk_d[:].rearrange("(c p) e -> p c e", p=128))
    ps_k0 = opps.tile([128, TB], f32, name=f"{R}k0s0ps", tag="op0")
    ps_q0 = opps.tile([128, TB], f32, name=f"{R}q0t0ps", tag="op1")
    kmm = [lambda k=k: nc.tensor.matmul(
        ps_k0[:], wk_t[:, k, 0:128], hT_t[:, k, 0:TB],
        start=(k == 0), stop=(k == KC - 1)) for k in range(KC)]
    qmm = [lambda k=k: nc.tensor.matmul(
        ps_q0[:], wq_t[:, k, 0:128], hT_t[:, k, 0:TB],
        start=(k == 0), stop=(k == KC - 1)) for k in range(KC)]
    for k in range(2):
        nc.sync.dma_start(hT_t[:, k, 0:TB], hT_d[k * 128:(k + 1) * 128, 0:TB])
        kmm[k]()
    nc.sync.dma_start(
        wq_t[:], wq_d[:].rearrange("(c p) e -> p c e", p=128))
    for k in range(2, KC):
        nc.sync.dma_start(hT_t[:, k, 0:TB], hT_d[k * 128:(k + 1) * 128, 0:TB])
        kmm[k]()
        qmm[k - 2]()
    qmm[KC - 2]()
    qmm[KC - 1]()
    nc.sync.dma_start(bk_t[:], bk_d[:].rearrange("c p o -> p c o"))
    nc.sync.dma_start(bq_t[:], bq_d[:].rearrange("c p o -> p c o"))
    nc.sync.dma_start(bv_t[:], bv_d[:])
    # k-drain on DVE, q-drain on the (idle at startup) ACT engine so the
    # two don't serialize ahead of the first scores
    nc.vector.tensor_scalar(kT[0][:, 0:TB], ps_k0[:], bk_t[:, 0, :], None,
                            Add)
    nc.scalar.activation(qT[0][:, 0:TB], ps_q0[:],
                         mybir.ActivationFunctionType.Identity,
                         bias=bq_t[:, 0, :])

    def _hq(qd):
        nc.sync.dma_start(
            hT_t[:, :, qd * TB:(qd + 1) * TB],
            hT_d[:, qd * TB:(qd + 1) * TB].rearrange("(c p) t -> p c t",
                                                     p=128))

    def _xq(qd):
        nc.sync.dma_start(
            xT_t[:, :, qd * TB:(qd + 1) * TB],
            xT_d[:, qd * TB:(qd + 1) * TB].rearrange("(c p) t -> p c t",
                                                     p=128))

    _hq(1)
    _hq(2)
    _hq(3)
    nc.sync.dma_start(
        wv_t[:], wv_d[:].rearrange("(c p) e -> p c e", p=128))
    _xq(0)
    _xq(1)
    _xq(2)
    _xq(3)
    nc.sync.dma_start(
        wo_t[:], wo_d[:].rearrange("(c p) d -> p c d", p=128))

    # ---- filler queue: deadline-ordered projection / out_proj work ----
    tag_i = [0]

    def next_tag():
        t = f"op{tag_i[0] % 2}"
        tag_i[0] += 1
        return t

    def qk_group(w_t, b_t, dstT, p, blk, nm):
        box = {}
        tag = [None]

        def mk_mm(k):
            def f():
                if k == 0:
                    tag[0] = next_tag()
                    box["ps"] = opps.tile([128, TB], f32,
                                          name=f"{R}{nm}ps", tag=tag[0])
                nc.tensor.matmul(box["ps"][:],
                                 w_t[:, k, p * 128:(p + 1) * 128],
                                 hT_t[:, k, blk * TB:(blk + 1) * TB],
                                 start=(k == 0), stop=(k == KC - 1))
            return f

        ops = [(213, mk_mm(k)) for k in range(KC)]

        def drain():
            nc.vector.tensor_scalar(dstT[p][:, blk * TB:(blk + 1) * TB],
                                    box["ps"][:], b_t[:, p, :], None, Add)
        ops.append((0, drain))
        return ops

    def v_group(j):
        # si pair (2j, 2j+1): two 8-matmul chains into one psum bank
        box = {}
        tag = [None]

        def mk_mm(k, jj):
            def f():
                if k == 0 and jj == 0:
                    tag[0] = next_tag()
                    box["ps"] = opps.tile([128, 2, E], f32,
                                          name=f"{R}v{j}ps", tag=tag[0])
                si = 2 * j + jj
                # HW: start=True zeroes the whole psum bank, so only the
                # first chain in the bank starts; the sibling accumulates.
                nc.tensor.matmul(box["ps"][:, jj, :],
                                 xT_t[:, k, si * 128:(si + 1) * 128],
                                 wv_t[:, k, :],
                                 start=(k == 0 and jj == 0),
                                 stop=(k == KC - 1),
                                 skip_group_check=True)
            return f

        # chain jj=0 completes (and drains) first so the attnV that
        # consumes v(2j) is not gated on the whole 16-matmul group
        ops = [(107, mk_mm(0, 0)), (107, mk_mm(0, 1))]
        for k in range(1, KC):
            ops.append((107, mk_mm(k, 0)))

        def mk_drain(jj):
            def f():
                si = 2 * j + jj
                nc.vector.tensor_tensor(
                    v_aug[:, si, :, 0:64],
                    box["ps"][:, jj, :].rearrange("p (g e) -> p g e", g=G),
                    bv_t[:].rearrange("p (g e) -> p g e", g=G),
                    op=Add)
            return f
        ops.append((0, mk_drain(0)))
        for k in range(1, KC):
            ops.append((107, mk_mm(k, 1)))
        ops.append((0, mk_drain(1)))
        return ops

    def out_unit(tb, ts, act_drain=False):
        # one 128-row output slice: dc0 fills one psum bank, dc1 the
        # other; both drain into one bf16 staging row -> single DMA.
        box = {}

        def mk_mm(dc, p):
            def f():
                if p == 0:
                    box[dc] = opps.tile([128, TB], f32,
                                        name=f"{R}o{tb}_{ts}_{dc}ps",
                                        tag=next_tag())
                nc.tensor.matmul(box[dc][:],
                                 at2[p][:, tb * TB + ts * 128:
                                        tb * TB + ts * 128 + 128],
                                 wo_t[:, p, dc * TB:(dc + 1) * TB],
                                 start=(p == 0), stop=(p == 1))
            return f

        def mk_drain(dc):
            def f():
                if dc == 0:
                    box["osb"] = ospool.tile([128, D], bf16,
                                             name=f"{R}o{tb}_{ts}sb",
                                             tag="os")
                if dc == 1 and act_drain:
                    nc.scalar.activation(
                        box["osb"][:, dc * TB:(dc + 1) * TB], box[dc][:],
                        mybir.ActivationFunctionType.Identity)
                else:
                    nc.vector.tensor_copy(
                        box["osb"][:, dc * TB:(dc + 1) * TB], box[dc][:])
            return f

        def dma():
            nc.sync.dma_start(
                out_d[tb * TB + ts * 128: tb * TB + (ts + 1) * 128, :],
                box["osb"][:])
        return [(213, mk_mm(0, 0)), (213, mk_mm(0, 1)), (0, mk_drain(0)),
                (213, mk_mm(1, 0)), (213, mk_mm(1, 1)), (0, mk_drain(1)),
                (0, dma)]

    fillers = []
    filler_by_key = {}

    def filler_item(ready, ops, key=None):
        it = {"ready": ready, "ops": ops, "i": 0, "key": key}
        fillers.append(it)
        if key is not None:
            filler_by_key[key] = it

    def ensure(key, _dbg=[0]):
        """Force-emit every remaining op of the filler item `key` so a
        consumer emitted next observes its writes (tile deps only order
        instructions that are already emitted)."""
        it = filler_by_key.get(key)
        if it is None:
            return
        n = len(it["ops"]) - it["i"]
        if n > 0 and _DBG:
            print(f"ENSURE {key} forces {n} ops at step {CUR_STEP[0]}")
        while it["i"] < len(it["ops"]):
            _, fn = it["ops"][it["i"]]
            it["i"] += 1
            fn()

    # deadline-ordered: (the scan picks the first *ready* item)
    filler_item(1, qk_group(wk_t, bk_t, kT, 0, 1, "k0s1"), ("k", 0, 1))
    filler_item(4, qk_group(wk_t, bk_t, kT, 0, 2, "k0s2"), ("k", 0, 2))
    filler_item(7, qk_group(wk_t, bk_t, kT, 0, 3, "k0s3"), ("k", 0, 3))
    filler_item(2, qk_group(wq_t, bq_t, qT, 0, 1, "q0t1"), ("q", 0, 1))
    filler_item(4, qk_group(wq_t, bq_t, qT, 0, 2, "q0t2"), ("q", 0, 2))
    filler_item(11, v_group(0), ("v", 0))
    filler_item(11, v_group(1), ("v", 1))
    filler_item(14, v_group(2), ("v", 2))
    filler_item(14, v_group(3), ("v", 3))
    filler_item(16, v_group(4), ("v", 4))
    filler_item(16, v_group(5), ("v", 5))
    filler_item(18, v_group(6), ("v", 6))
    filler_item(18, v_group(7), ("v", 7))
    filler_item(7, qk_group(wq_t, bq_t, qT, 0, 3, "q0t3"), ("q", 0, 3))
    filler_item(44, qk_group(wk_t, bk_t, kT, 1, 0, "k1s0"), ("k", 1, 0))
    filler_item(47, qk_group(wq_t, bq_t, qT, 1, 0, "q1t0"), ("q", 1, 0))
    filler_item(50, qk_group(wk_t, bk_t, kT, 1, 1, "k1s1"), ("k", 1, 1))
    filler_item(53, qk_group(wk_t, bk_t, kT, 1, 2, "k1s2"), ("k", 1, 2))
    filler_item(56, qk_group(wk_t, bk_t, kT, 1, 3, "k1s3"), ("k", 1, 3))
    filler_item(60, qk_group(wq_t, bq_t, qT, 1, 1, "q1t1"), ("q", 1, 1))
    filler_item(64, qk_group(wq_t, bq_t, qT, 1, 2, "q1t2"), ("q", 1, 2))
    filler_item(68, qk_group(wq_t, bq_t, qT, 1, 3, "q1t3"), ("q", 1, 3))

    def run_fillers(step, budget):
        spent = 0
        while spent < budget:
            it = None
            for x in fillers:
                if x["i"] < len(x["ops"]) and x["ready"] <= step:
                    it = x
                    break
            if it is None:
                return
            cost, fn = it["ops"][it["i"]]
            it["i"] += 1
            fn()
            spent += cost

    # ---- attention machinery ----
    CUR_STEP = [0]
    ex_store = {}
    atp_store = {}
    a2n_store = {}

    def scores_exp(pi, si):
        tb, p = PAIRS[pi]
        ensure(("k", p, si // 4))
        ensure(("q", p, tb))
        scp = scps.tile([128, 2, TB], f32, name=f"{R}sc{pi}_{si}", tag="sc")
        for h in range(2):
            nc.tensor.matmul(
                scp[:, h, :],
                kT[p][h * 64:(h + 1) * 64, si * 128:(si + 1) * 128],
                qT[p][h * 64:(h + 1) * 64, tb * TB:(tb + 1) * TB],
                start=True, stop=True)
        ex = expool.tile([128, 2, TB], bf16, name=f"{R}ex{pi}_{si}", tag="ex")
        if pi == len(PAIRS) - 1 and si == NS - 1:
            # the very last exp gates the tail: split by head so the h0
            # attnV chains (and everything after) start half an exp early
            nc.scalar.activation(ex[:, 0, :], scp[:, 0, :], Exp)
            nc.scalar.activation(ex[:, 1, :], scp[:, 1, :], Exp)
        else:
            nc.scalar.activation(ex[:], scp[:], Exp)
        ex_store[(pi, si)] = ex

    def attnv(pi, si):
        tb, p = PAIRS[pi]
        ensure(("v", si // 2))
        if si == 0:
            atp_store[pi] = atps.tile([128, 8, 128], f32,
                                      name=f"{R}atp{pi}", tag="at")
        atp = atp_store[pi]
        ex = ex_store.pop((pi, si))
        for h in range(2):
            for tcn in range(TCN):
                u = h * TCN + tcn
                # start=True zeroes the whole bank on HW: chains u=0..3
                # live in bank A (zeroed by u==0), u=4..7 in bank B
                # (zeroed by u==4); all siblings accumulate.
                nc.tensor.matmul(
                    atp[:, u, 0:65],
                    ex[:, h, tcn * 128:(tcn + 1) * 128],
                    v_aug[:, si, p * 2 + h, 0:65],
                    start=(si == 0 and u % 4 == 0), stop=(si == NS - 1),
                    skip_group_check=True)

    def normalize(pi):
        from concourse.bass import broadcast_tensor_aps
        tb, p = PAIRS[pi]
        atp = atp_store.pop(pi)
        a2n = a2pool.tile([128, TCN, 128], bf16, name=f"{R}a2n{pi}",
                          tag="a2n")
        nc.vector.reciprocal(rec8[:], atp[:, :, 64:65])
        # all 8 (h, tc) slots normalized in ONE DVE op: the reciprocal
        # column broadcasts over e via a stride-0 AP
        av = atp[:, :, 0:64].rearrange("p (h c) e -> p c h e", h=2)
        rv = rec8[:].rearrange("p (h c) o -> p c h o", h=2)
        av2, rv2 = broadcast_tensor_aps(av, rv)
        nc.vector.tensor_tensor(
            a2n[:].rearrange("p c (h e) -> p c h e", h=2), av2, rv2,
            op=Mult)
        a2n_store[pi] = a2n

    def dma_transpose(pi):
        tb, p = PAIRS[pi]
        a2n = a2n_store.pop(pi)
        for tcn in range(TCN):
            nc.sync.dma_start_transpose(
                at2[p][:, tb * TB + tcn * 128: tb * TB + (tcn + 1) * 128],
                a2n[:, tcn, :])

    # ---- main pipeline ----
    for pi in range(len(PAIRS)):
        for si in range(NS):
            gs = pi * NS + si
            CUR_STEP[0] = gs
            scores_exp(pi, si)
            # deferred attnV runs one step late within the pair (chains
            # have a whole pair of slack), so the pair-boundary atp WAR
            # on the previous normalize resolves during step 0
            if pi == len(PAIRS) - 1:
                # last pair: drain the previous pair's attnV at double
                # rate, normalize it mid-pair, then chase this pair's own
                # attnV so the tail is short.
                if si == 0:
                    budget = 380
                elif si <= 8:
                    attnv(pi - 1, 2 * (si - 1))
                    attnv(pi - 1, 2 * si - 1)
                    budget = 220
                    if si == 8:
                        normalize(pi - 1)
                        dma_transpose(pi - 1)
                        # half of the second-to-last tb's out_proj runs
                        # here; the rest fills the tail's dead PE time
                        tb_p = PAIRS[pi - 1][0]
                        for ts in range(2):
                            filler_item(gs + 1, out_unit(tb_p, ts))
                        budget = 300
                else:  # si 9..15: emit attnV(pi) for si 0..2*(si-9)+1
                    attnv(pi, 2 * (si - 9))
                    attnv(pi, 2 * (si - 9) + 1)
                    if si == NS - 1:
                        attnv(pi, 14)
                    budget = 300
            elif pi > 0:
                if si == 0:
                    budget = 380
                elif si == NS - 1:
                    attnv(pi - 1, si - 1)
                    attnv(pi - 1, si)
                    budget = 300
                else:
                    attnv(pi - 1, si - 1)
                    budget = 380
            else:
                budget = 750 if si < 6 else 650
            run_fillers(gs, budget)
        if 0 < pi < len(PAIRS) - 1:
            normalize(pi - 1)
            dma_transpose(pi - 1)
            if PAIRS[pi - 1][1] == 1:
                # at2 for this tb is now complete on both pairs ->
                # out-projection becomes available filler work
                tb = PAIRS[pi - 1][0]
                for ts in range(TCN):
                    filler_item(pi * NS + 1, out_unit(tb, ts))

    # ---- tail: finish last pair per t-chunk, ACT helps with drains ----
    Iden = mybir.ActivationFunctionType.Identity
    last = len(PAIRS) - 1
    tb3 = PAIRS[last][0]
    attnv(last, NS - 1)
    from concourse.bass import broadcast_tensor_aps
    atp = atp_store.pop(last)
    a2n = a2pool.tile([128, TCN, 128], bf16, name=f"{R}a2nT", tag="a2n")
    nc.vector.reciprocal(rec8[:], atp[:, :, 64:65])
    av = atp[:, :, 0:64].rearrange("p (h c) e -> p c h e", h=2)
    rv = rec8[:].rearrange("p (h c) o -> p c h o", h=2)
    av2, rv2 = broadcast_tensor_aps(av, rv)
    nc.vector.tensor_tensor(
        a2n[:].rearrange("p c (h e) -> p c h e", h=2), av2, rv2, op=Mult)
    run_fillers(10 ** 9, 10 ** 9)  # stragglers
    # the deferred half of out(tb2): its fills overlap the normalize
    for ts in (2, 3):
        for _, fn in out_unit(PAIRS[-2][0], ts):
            fn()
    # PE-transpose into psum (53ns each) + copy instead of DMA-transpose:
    # saves the ~2.3us DGE/sem latency on the tail critical path.
    # Phase-ordered emission (all transposes -> all copies -> fills with
    # drains/DMAs chasing) so the in-order PE queue never interleaves a
    # stalled op ahead of ready fills.
    tp = atps.tile([128, TCN, 128], bf16, name=f"{R}tpT", tag="at")
    for tcn in range(TCN):
        nc.tensor.matmul(tp[:, tcn, :], a2n[:, tcn, :], id_t[:],
                         is_transpose=True, start=(tcn == 0), stop=True,
                         skip_group_check=True)
    for tcn in range(TCN):
        at2s = at2[1][:, tb3 * TB + tcn * 128: tb3 * TB + (tcn + 1) * 128]
        if tcn % 2 == 0:
            nc.vector.tensor_copy(at2s, tp[:, tcn, :])
        else:
            nc.scalar.activation(at2s, tp[:, tcn, :], Iden)
    psds, osbs = [], []
    for tcn in range(TCN):
        if tcn % 2 == 0:
            psd = [opps.tile([128, TB], f32, name=f"{R}ot{tcn}_{dc}",
                             tag=next_tag()) for dc in range(2)]
        else:
            scpair = scps.tile([128, 2, TB], f32, name=f"{R}ot{tcn}",
                               tag="sc")
            psd = [scpair[:, 0, :], scpair[:, 1, :]]
        psds.append(psd)
        osbs.append(ospool.tile([128, D], bf16, name=f"{R}ot{tcn}sb",
                                tag="os"))
    for tcn in range(TCN):
        for dc in range(2):
            for p in range(2):
                nc.tensor.matmul(
                    psds[tcn][dc][:],
                    at2[p][:, tb3 * TB + tcn * 128:
                           tb3 * TB + tcn * 128 + 128],
                    wo_t[:, p, dc * TB:(dc + 1) * TB],
                    start=(p == 0), stop=(p == 1))
            if dc == 0:
                nc.vector.tensor_copy(osbs[tcn][:, 0:TB], psds[tcn][0][:])
            else:
                nc.scalar.activation(osbs[tcn][:, TB:D], psds[tcn][1][:],
                                     Iden)
        nc.sync.dma_start(
            out_d[tb3 * TB + tcn * 128: tb3 * TB + (tcn + 1) * 128, :],
            osbs[tcn][:])

    octx.close()


def _get_program(reps=1):
    global _PROGRAM
    if _PROGRAM is None:
        _PROGRAM = {}
    if reps not in _PROGRAM:
        _PROGRAM[reps] = _build_program(reps)
    return _PROGRAM[reps]


def _shard_inputs(inputs):
    """Build the 8 per-core input maps from the full-problem inputs."""
    bf16 = ml_dtypes.bfloat16
    hs = np.asarray(inputs["hidden_states"], np.float32)
    pe = np.asarray(inputs["position_embeddings"], np.float32)
    Wq = np.asarray(inputs["Wq"], np.float32).reshape(D, H * HD)
    Wk = np.asarray(inputs["Wk"], np.float32).reshape(D, H * HD)
    Wv = np.asarray(inputs["Wv"], np.float32).reshape(D, H * HD)
    Wo = np.asarray(inputs["Wo"], np.float32)
    bq = np.asarray(inputs["bq"], np.float32).reshape(H * HD)
    bk = np.asarray(inputs["bk"], np.float32).reshape(H * HD)
    bv = np.asarray(inputs["bv"], np.float32).reshape(H * HD)

    h = hs + pe
    hT = [np.ascontiguousarray(h[b].T).astype(bf16) for b in range(B)]
    xT = [np.ascontiguousarray(hs[b].T).astype(bf16) for b in range(B)]

    in_maps = []
    for c in range(8):
        b, g = divmod(c, G)
        sel = slice(g * E, (g + 1) * E)
        in_maps.append({
            "hT": hT[b],
            "xT": xT[b],
            "wq": (np.ascontiguousarray(Wq[:, sel])
                   * np.float32(SCALE)).astype(bf16),
            "wk": np.ascontiguousarray(Wk[:, sel]).astype(bf16),
            "wv": np.ascontiguousarray(Wv[:, sel]).astype(bf16),
            "wo": np.ascontiguousarray(Wo[sel, :]).astype(bf16),
            "bq": (bq[sel] * np.float32(SCALE)).reshape(2, 128, 1).copy(),
            "bk": bk[sel].reshape(2, 128, 1).copy(),
            "bvr": np.tile(bv[sel][None, :], (128, 1)),
        })
    return in_maps


def _gather_outputs(results, inputs):
    bo = np.asarray(inputs["bo"], np.float32)
    out = np.empty((B, S, D), np.float32)
    for b in range(B):
        acc = results[4 * b]["out"].astype(np.float32).copy()
        for g in range(1, G):
            acc += results[4 * b + g]["out"]
        out[b] = acc + bo[None, :]
    return out


def kernel(**inputs):
    from concourse.bass_utils import run_bass_kernel_spmd

    nc = _get_program()
    in_maps = _shard_inputs(inputs)
    res = run_bass_kernel_spmd(nc, in_maps, list(range(8)))
    return _gather_outputs(res.results, inputs)
